# revision 74
# baseline (speedup 1.0000x reference)
"""Trainium2 Bass kernel for nn_Block_84155589198355 (dense transformer block).

Strategy: pure data parallelism — B=8 batch elements over 8 NeuronCores, one
full transformer block per core (no collectives). Heavy matmuls run in
fp8(e4m3) DoubleRow perf mode (two 128-deep contraction slots per
instruction, 0.5 PE cycles/row = 4x bf16 throughput), with precision managed
per stage against the 2e-2 rel-err budget:

  - residual stream x kept in bf16 (host-cast), stats/psum accumulation fp32
  - weights pre-scaled by 32 (W2 by 64) to unit std and split host-side into
    e4m3 (hi, lo) pairs; psum scale undone at evict (gelu scale=1/32 etc.)
  - Q/K/V/proj: both operands native e4m3 (kc-paired DoubleRow)
  - MLP1/MLP2: both operands hi+lo ("ss": hh+lh+hl terms) — hT and gelu
    output split on-chip (gelu -> bf16 scratch; hi cast on Pool, lo on DVE)
  - S^T = K^T q: k split into (k_hi, k_lo) slots vs q duplicated across both
    slots (only stage needing duplication; dup copy on Pool)
  - AV: E (exp, shifted by ESHIFT so e4m3 never overflows; shift cancels in
    softmax) and V native e4m3, kt-paired DoubleRow; ones column in V gives
    the softmax denominator; causal triangle applied post-exp as a 0/1
    mask-multiply on Pool (gpsimd)

Schedule: qc0 attention first (V t4..7 deferred into its exp-bound stream),
then qc1 with proj/LN2 for t0..3 plus 24 MLP1 n2=0 chunks (raw psums parked
in a bf16 u0 scratch, gelu deferred past attention) as PE filler; w1 weights
for the first post-attention MLP1 iterations prefetch while the DMA engines
are idle late in attention. g is hi+lo split only for the second half of the
FF dimension (first half native e4m3 straight from ACT gelu — no cast/sub
and no lo-term in MLP2), trading ~0.4e-2 rel err for ~25us. LN mean for LN2
comes free from the proj-evict accumulator; rsqrt via bit-trick Newton on
DVE; output written bf16 (host upcasts).
"""

import sys

if "/opt/trn_rl_repo" not in sys.path:
    sys.path.insert(0, "/opt/trn_rl_repo")

import numpy as np
import ml_dtypes

B, T, C, H = 8, 1024, 1024, 16
D = C // H
FF = 4 * C
P = 128
NT = T // P      # 8 token tiles
NKC = C // P     # 8 contraction chunks over C
NM = FF // P     # 32 chunks over FF
COND_LEN = 256
TOKEN_LEN = 768
NEG = -1.0e9
EPS = 1e-5
ESHIFT = 3.0  # logit shift so exp output fits e4m3 (cancels in softmax)
BF16 = ml_dtypes.bfloat16
E4M3 = ml_dtypes.float8_e4m3


def _q8(x):
    return x.astype(E4M3)


def _split8(x, scale):
    """Return (hi, lo) e4m3 pair with hi at `scale`*x; lo at same scale."""
    xs = x * scale
    hi = _q8(xs)
    lo = _q8(xs - hi.astype(np.float32))
    return hi, lo

_BUILD_CACHE = {}


def _build(flags):
    """Build and compile the per-core Bass program. flags is a tuple of bools:
    (qk_bias, v_bias, p_bias, b1_bias, b2_bias, ln1_aff, ln2_aff)."""
    import concourse.bass as bass
    from concourse import bacc, tile, mybir

    qk_bias, v_bias, p_bias, b1_bias, b2_bias, ln1_aff, ln2_aff = flags
    f32 = mybir.dt.float32
    i32 = mybir.dt.int32
    bf16 = mybir.dt.bfloat16
    AF = mybir.ActivationFunctionType
    OP = mybir.AluOpType
    AX = mybir.AxisListType

    nc = bacc.Bacc("TRN2", target_bir_lowering=False, debug=False)

    fp8 = mybir.dt.float8e4
    DR = mybir.MatmulPerfMode.DoubleRow
    x_d = nc.dram_tensor("x", [T, C], bf16, kind="ExternalInput")
    qkvp_d = {}
    for wn in ("wq", "wk", "wv", "wp"):
        for hl in ("h", "l"):
            qkvp_d[wn + hl] = nc.dram_tensor(
                wn + hl, [C, C], fp8, kind="ExternalInput"
            )
    w1h_d = nc.dram_tensor("w1h", [C, FF], fp8, kind="ExternalInput")
    w1l_d = nc.dram_tensor("w1l", [C, FF], fp8, kind="ExternalInput")
    w2h_d = nc.dram_tensor("w2h", [FF, C], fp8, kind="ExternalInput")
    w2l_d = nc.dram_tensor("w2l", [FF, C], fp8, kind="ExternalInput")
    cb_d = nc.dram_tensor("cbias", [P, 3], f32, kind="ExternalInput")
    tri_d = nc.dram_tensor("tri", [P, P], fp8, kind="ExternalInput")
    id_d = nc.dram_tensor("ident", [P, P], bf16, kind="ExternalInput")
    out_d = nc.dram_tensor("out", [T, C], bf16, kind="ExternalOutput")

    opt_d = {}
    if qk_bias:
        opt_d["bq"] = nc.dram_tensor("bq", [P, NKC], f32, kind="ExternalInput")
        opt_d["bk"] = nc.dram_tensor("bk", [P, NKC], f32, kind="ExternalInput")
    if v_bias:
        opt_d["bv"] = nc.dram_tensor("bv", [1, C], bf16, kind="ExternalInput")
    if p_bias:
        opt_d["bp"] = nc.dram_tensor("bp", [1, C], bf16, kind="ExternalInput")
    if b1_bias:
        opt_d["b1"] = nc.dram_tensor("b1", [P, NM], f32, kind="ExternalInput")
    if b2_bias:
        opt_d["b2"] = nc.dram_tensor("b2", [1, C], bf16, kind="ExternalInput")
    if ln1_aff:
        opt_d["g1"] = nc.dram_tensor("g1", [P, C], f32, kind="ExternalInput")
        opt_d["o1"] = nc.dram_tensor("o1", [P, C], f32, kind="ExternalInput")
    if ln2_aff:
        opt_d["g2"] = nc.dram_tensor("g2", [P, C], f32, kind="ExternalInput")
        opt_d["o2"] = nc.dram_tensor("o2", [P, C], f32, kind="ExternalInput")

    x_re = x_d.ap().rearrange("(t p) c -> p t c", p=P)
    out_re = out_d.ap().rearrange("(t p) c -> p t c", p=P)
    qkvp_re = {
        nm: d.ap().rearrange("(k p) m -> p k m", p=P)
        for nm, d in qkvp_d.items()
    }
    w1h_re = w1h_d.ap().rearrange("(k p) m -> p k m", p=P)
    w1l_re = w1l_d.ap().rearrange("(k p) m -> p k m", p=P)
    w2h_re = w2h_d.ap().rearrange("(k p) m -> p k m", p=P)
    w2l_re = w2l_d.ap().rearrange("(k p) m -> p k m", p=P)

    def kts_for(qc):
        # visible k-tiles for q-chunk qc (512-wide chunks)
        return range(4) if qc == 0 else range(8)

    with tile.TileContext(nc) as tc:
        import contextlib

        with contextlib.ExitStack() as ctx:
            cpool = ctx.enter_context(tc.tile_pool(name="const", bufs=1))
            xpool = ctx.enter_context(tc.tile_pool(name="xres", bufs=1))
            apool = ctx.enter_context(tc.tile_pool(name="act", bufs=1))
            spool = ctx.enter_context(tc.tile_pool(name="small", bufs=8))
            sqpool = ctx.enter_context(tc.tile_pool(name="sqscr", bufs=1))
            # one shared [128,512] fp32 psum tag for QKV / S^T / proj / MLP2 —
            # avoids pool-boundary serialization between phases
            mmps = ctx.enter_context(
                tc.tile_pool(name="mm512", bufs=5, space="PSUM")
            )
            # w1 stream pool lives at top level so its first DMAs aren't
            # gated on the attention-phase pools releasing SBUF
            w1p = ctx.enter_context(tc.tile_pool(name="w1p", bufs=9))
            w1pre = ctx.enter_context(tc.tile_pool(name="w1pre", bufs=1))
            u0p = ctx.enter_context(tc.tile_pool(name="u0", bufs=1))

            tri_sb = cpool.tile([P, P], fp8, tag="tri")
            nc.sync.dma_start(tri_sb[:], tri_d[:])
            id_sb = cpool.tile([P, P], bf16, tag="ident")
            nc.sync.dma_start(id_sb[:], id_d[:])
            cb_sb = cpool.tile([P, 3], f32, tag="cbias")
            nc.sync.dma_start(cb_sb[:], cb_d[:])
            magic_sb = cpool.tile([P, 1], i32, tag="magic")
            nc.vector.memset(magic_sb[:], 0x5F3759DF)
            need_ones_b = v_bias or p_bias or b2_bias
            if need_ones_b:
                ones_b = cpool.tile([1, P], bf16, tag="onesb")
                nc.gpsimd.memset(ones_b[:], 1.0)
            opt_sb = {}
            for nm, dd in opt_d.items():
                shp = list(dd.shape)
                dt_ = dd.dtype
                opt_sb[nm] = cpool.tile(shp, dt_, tag=nm)
                nc.sync.dma_start(opt_sb[nm][:], dd[:])

            x_sb = xpool.tile([P, NT, C], bf16, tag="x")
            for t in range(4):
                nc.sync.dma_start(x_sb[:, t, :], x_re[:, t, :])

            # ---------------- LayerNorm (token-major) + transpose ----------
            def ln_tile(dst_tok, t, affine, act_mean=False, mean_acc=None,
                        norm_act=False, act_sq=False):
                    xr = x_sb[:, t, :]
                    mu = spool.tile([P, 1], f32, tag="mu")
                    if mean_acc is not None:
                        # row-sums already accumulated by the residual-evict
                        nc.vector.tensor_add(
                            mu, mean_acc[:, 0:1], mean_acc[:, 1:2]
                        )
                        nc.vector.tensor_scalar_mul(mu, mu, 1.0 / C)
                    elif act_mean:
                        # mean via ACT Copy+accum (frees DVE on the startup
                        # critical path; Copy shares exp's LUT set)
                        cs = sqpool.tile([P, C], bf16, tag="sq")
                        nc.scalar.activation(cs, xr, AF.Copy, accum_out=mu)
                        nc.vector.tensor_scalar_mul(mu, mu, 1.0 / C)
                    else:
                        nc.vector.tensor_reduce(mu, xr, axis=AX.X, op=OP.add)
                        nc.vector.tensor_scalar_mul(mu, mu, 1.0 / C)
                    sq = sqpool.tile([P, C], bf16, tag="sq")
                    ss = spool.tile([P, 1], f32, tag="ss")
                    if mean_acc is not None and not act_sq:
                        # qc1-window LN: keep ACT free for exp — square on DVE
                        nc.vector.scalar_tensor_tensor(
                            sq, xr, 1.0, xr, op0=OP.mult, op1=OP.mult,
                            accum_out=ss,
                        )
                    else:
                        nc.scalar.activation(sq, xr, AF.Square, accum_out=ss)
                    var = spool.tile([P, 1], f32, tag="var")
                    musq = spool.tile([P, 1], f32, tag="musq")
                    nc.vector.tensor_mul(musq, mu, mu)
                    nc.vector.tensor_scalar_mul(var, ss, 1.0 / C)
                    nc.vector.tensor_sub(var, var, musq)
                    nc.vector.tensor_scalar_add(var, var, EPS)
                    # rstd = rsqrt(var) on DVE (bit-trick + 3 Newton steps):
                    # ACT Sqrt/Ln would thrash LUT-table loads against the
                    # attention exp stream (different act_func_sets)
                    rstd = spool.tile([P, 1], f32, tag="rstd")
                    ri = rstd[:].bitcast(i32)
                    nc.vector.tensor_single_scalar(
                        ri, var[:].bitcast(i32), 1, op=OP.arith_shift_right
                    )
                    nc.vector.tensor_sub(ri, magic_sb[:], ri)
                    nsq = spool.tile([P, 1], f32, tag="nsq")
                    for _ in range(2):
                        nc.vector.tensor_mul(nsq, rstd, rstd)
                        nc.vector.tensor_mul(nsq, nsq, var)
                        nc.vector.tensor_scalar(
                            nsq, nsq, -0.5, 1.5, op0=OP.mult, op1=OP.add
                        )
                        nc.vector.tensor_mul(rstd, rstd, nsq)
                    if affine is None and norm_act:
                        # xn = Identity(x*rstd + (-mu*rstd)) on ACT — used on
                        # alternate LN1 tiles to split the normalize pass
                        # across both engines (Identity shares exp's LUT set)
                        nmr = spool.tile([P, 1], f32, tag="nmr")
                        nc.vector.tensor_mul(nmr, mu, rstd)
                        nc.vector.tensor_scalar_mul(nmr, nmr, -1.0)
                        nc.scalar.activation(
                            dst_tok[:, t, :], xr, AF.Identity,
                            bias=nmr, scale=rstd,
                        )
                    elif affine is None:
                        nc.vector.tensor_scalar(
                            dst_tok[:, t, :], xr, mu, rstd,
                            op0=OP.subtract, op1=OP.mult,
                        )
                    else:
                        g_sb_, o_sb_ = affine
                        tmp = spool.tile([P, C], f32, tag="lntmp")
                        nc.vector.tensor_scalar(
                            tmp, xr, mu, rstd, op0=OP.subtract, op1=OP.mult
                        )
                        nc.vector.tensor_mul(tmp, tmp, g_sb_[:])
                        nc.vector.tensor_add(dst_tok[:, t, :], tmp, o_sb_[:])

            def transp_tile(dst_fT, src_tok, t, psum_pool, tag="tp"):
                for mc in range(NKC):
                    tp = psum_pool.tile([P, P], bf16, tag=tag,
                                        name=f"tp{t}_{mc}")
                    nc.tensor.transpose(
                        tp, src_tok[:, t, mc * P:(mc + 1) * P], id_sb[:]
                    )
                    nc.vector.tensor_copy(
                        dst_fT[:, mc, t * P:(t + 1) * P], tp
                    )

            def transp_one(dst_h, src_tok, t, psum_pool, tag="tp"):
                # batched transpose, single e4m3 evict (no lo residual)
                for half in range(2):
                    mc0 = half * 4
                    tp = psum_pool.tile([P, 512], bf16, tag=tag,
                                        name=f"t1_{t}_{half}")
                    for i in range(4):
                        nc.tensor.transpose(
                            tp[:, i * P:(i + 1) * P],
                            src_tok[:, t, (mc0 + i) * P:(mc0 + i + 1) * P],
                            id_sb[:],
                        )
                    nc.vector.tensor_copy(
                        dst_h[:, mc0:mc0 + 4, t * P:(t + 1) * P],
                        tp.rearrange("p (k q) -> p k q", q=P),
                    )

            def transp_split(dst_h, dst_l, src_tok, t, psum_pool, tag="tp"):
                # transpose 4 feature blocks into one [P,512] psum, then
                # evict as e4m3 hi + lo (lo = exact - hi)
                for half in range(2):
                    mc0 = half * 4
                    tp = psum_pool.tile([P, 512], bf16, tag=tag,
                                        name=f"tsp{t}_{half}")
                    for i in range(4):
                        nc.tensor.transpose(
                            tp[:, i * P:(i + 1) * P],
                            src_tok[:, t, (mc0 + i) * P:(mc0 + i + 1) * P],
                            id_sb[:],
                        )
                    tp3 = tp.rearrange("p (k q) -> p k q", q=P)
                    hsl = dst_h[:, mc0:mc0 + 4, t * P:(t + 1) * P]
                    nc.vector.tensor_copy(hsl, tp3)
                    nc.vector.tensor_sub(
                        dst_l[:, mc0:mc0 + 4, t * P:(t + 1) * P], tp3, hsl
                    )

            # ---------------- QKV + attention + proj -----------------------
            # q8: [*, mc, 2(dup), T]; k8: [*, mc, 2(hi|lo), T] so the S^T
            # DoubleRow pairs (k_hi,q)+(k_lo,q). v8/y8 single e4m3.
            with contextlib.ExitStack() as actx:
                qkvy = actx.enter_context(tc.tile_pool(name="qkvy", bufs=1))
                wpool = actx.enter_context(tc.tile_pool(name="wstream", bufs=2))
                # wq streams ahead of the second x half so the first QK
                # matmuls aren't DMA-gated
                wq_th = wpool.tile([P, NKC, C], fp8, tag="wh")
                nc.sync.dma_start(wq_th[:], qkvp_re["wqh"])
                for t in range(4, NT):
                    nc.sync.dma_start(x_sb[:, t, :], x_re[:, t, :])

                ln1_args = (opt_sb["g1"][:], opt_sb["o1"][:]) if ln1_aff else None
                ln2_args = (opt_sb["g2"][:], opt_sb["o2"][:]) if ln2_aff else None
                xn_tok = apool.tile([P, NT, C], bf16, tag="tok")
                xnT_h = apool.tile([P, NKC, T], fp8, tag="xTh")
                with tc.tile_pool(name="tpsum", bufs=2, space="PSUM") as tpp:
                    for t in range(NT):
                        ln_tile(xn_tok, t, ln1_args)
                        transp_one(xnT_h, xn_tok, t, tpp)

                q8 = qkvy.tile([P, NKC, 2, T], fp8, tag="q")
                k8 = qkvy.tile([P, NKC, 2, T], fp8, tag="k")
                v8 = qkvy.tile([P, NT, H, D + 1], fp8, tag="v")
                y8 = qkvy.tile([P, NKC, T], fp8, tag="y")
                nc.vector.memset(v8[:, :, :, D:D + 1], 1.0)

                def mm_ss_dr(ps, terms, qsl_m, qsl_x, stop_ok=True):
                    """DoubleRow kc-paired products: terms = [(w, x), ...]"""
                    first = True
                    for ti, (wa, xa) in enumerate(terms):
                        for kp in range(NKC // 2):
                            nc.tensor.matmul(
                                ps,
                                wa[:, 2 * kp:2 * kp + 2, qsl_m],
                                xa[:, 2 * kp:2 * kp + 2, qsl_x],
                                start=first,
                                stop=stop_ok and (ti == len(terms) - 1)
                                and (kp == NKC // 2 - 1),
                                perf_mode=DR,
                            )
                            first = False

                # Q and K (feature-major); n2-outer so the first token half's
                # xnT transposes unblock matmuls early
                for which, dst in ((0, q8), (1, k8)):
                    if which == 0:
                        wht = wq_th
                    else:
                        wht = wpool.tile([P, NKC, C], fp8, tag="wh")
                        nc.sync.dma_start(wht[:], qkvp_re["wkh"])
                    for n2 in (0, 1):
                        for m in range(NKC):
                            ps = mmps.tile([P, 512], f32, tag="S")
                            qsl = slice(n2 * 512, (n2 + 1) * 512)
                            mm_ss_dr(
                                ps,
                                ((wht, xnT_h),),
                                slice(m * P, (m + 1) * P), qsl,
                            )
                            d0 = dst[:, m, 0, qsl]
                            d1 = dst[:, m, 1, qsl]
                            if qk_bias:
                                bias_nm = "bq" if which == 0 else "bk"
                                sc = sqpool.tile([P, 512], bf16, tag="qksc")
                                nc.scalar.activation(
                                    sc, ps, AF.Identity,
                                    bias=opt_sb[bias_nm][:, m:m + 1],
                                    scale=1.0 / 32,
                                )
                                nc.vector.tensor_copy(d0, sc)
                                if which == 0:
                                    nc.gpsimd.tensor_copy(d1, d0)
                                else:
                                    nc.vector.tensor_sub(d1, sc, d0)
                            else:
                                nc.scalar.activation(d0, ps, AF.Identity,
                                                     scale=1.0 / 32)
                                if which == 0:
                                    # q duplicated across both DR slots
                                    nc.gpsimd.tensor_copy(d1, d0)
                                else:
                                    # k_lo = exact - k_hi
                                    nc.vector.scalar_tensor_tensor(
                                        d1, ps, 1.0 / 32, d0,
                                        op0=OP.mult, op1=OP.subtract,
                                    )

                # V (token-major, strided into per-head 65-wide slots).
                # n2=0 (heads 0..7) now; n2=1 groups are deferred into the
                # ACT-bound qc1 attention stream as PE filler (heads 8..15
                # aren't consumed until the 9th qc1 pair).
                wvh_sb = wpool.tile([P, NKC, C], fp8, tag="wh")
                nc.sync.dma_start(wvh_sb[:], qkvp_re["wvh"])

                def emit_v(t, n2):
                    ps = mmps.tile([P, 512], f32, tag="S")
                    nsl = slice(n2 * 512, (n2 + 1) * 512)
                    tsl = slice(t * P, (t + 1) * P)
                    first = True
                    for kp in range(NKC // 2):
                        nc.tensor.matmul(
                            ps,
                            xnT_h[:, 2 * kp:2 * kp + 2, tsl],
                            wvh_sb[:, 2 * kp:2 * kp + 2, nsl],
                            start=first,
                            stop=(kp == NKC // 2 - 1) and not v_bias,
                            perf_mode=DR,
                        )
                        first = False
                    if v_bias:
                        nc.tensor.matmul(
                            ps, ones_b[:],
                            opt_sb["bv"][:, n2 * 512:(n2 + 1) * 512],
                            start=False, stop=True,
                        )
                    nc.scalar.activation(
                        v8[:, t, n2 * 8:(n2 + 1) * 8, 0:D],
                        ps.rearrange("p (h d) -> p h d", d=D),
                        AF.Identity, scale=1.0 / 32,
                    )

                for t in range(4):
                    emit_v(t, 0)
                    emit_v(t, 1)

                # ---- attention (qc0 first) with V t4..7 filling the qc0
                # stream and proj/LN2 for t0..3 filling the qc1 stream ----
                h_tok = apool.tile([P, NT, C], bf16, tag="tok")
                hT_h = apool.tile([P, NKC, T], fp8, tag="fTh")
                hT_l = apool.tile([P, NKC, T], fp8, tag="fTl")
                wph_sb = wpool.tile([P, NKC, C], fp8, tag="wh")
                nc.sync.dma_start(wph_sb[:], qkvp_re["wph"])

                def emit_proj(t, n2):
                    # proj is y8(native) @ Wp(hi+lo), mc-paired DoubleRow
                    ps = mmps.tile([P, 512], f32, tag="S")
                    nsl = slice(n2 * 512, (n2 + 1) * 512)
                    tsl = slice(t * P, (t + 1) * P)
                    first = True
                    for kp in range(NKC // 2):
                        nc.tensor.matmul(
                            ps,
                            y8[:, 2 * kp:2 * kp + 2, tsl],
                            wph_sb[:, 2 * kp:2 * kp + 2, nsl],
                            start=first,
                            stop=(kp == NKC // 2 - 1) and not p_bias,
                            perf_mode=DR,
                        )
                        first = False
                    if p_bias:
                        nc.tensor.matmul(
                            ps, ones_b[:],
                            opt_sb["bp"][:, n2 * 512:(n2 + 1) * 512],
                            start=False, stop=True,
                        )
                    xsl = x_sb[:, t, n2 * 512:(n2 + 1) * 512]
                    if t not in proj_acc:
                        proj_acc[t] = spool.tile([P, 2], f32, tag="pacc", name=f"pacc{t}")
                    # x1 = ps/32 + x, with the row-sum accumulated on the side
                    # so LN2 doesn't need its own mean-reduction pass
                    nc.vector.scalar_tensor_tensor(
                        xsl, ps, 1.0 / 32, xsl, op0=OP.mult, op1=OP.add,
                        accum_out=proj_acc[t][:, n2:n2 + 1],
                    )

                proj_acc = {}
                with (
                    tc.tile_pool(name="epool", bufs=2) as epool,
                    tc.tile_pool(name="attpy", bufs=2, space="PSUM") as yps,
                    tc.tile_pool(name="tpsum2", bufs=1, space="PSUM") as tp2,
                    tc.tile_pool(name="attsb", bufs=2) as asb,
                ):
                    e_tiles = {}

                    def emit_s_kt(h, qc, e_t, kt):
                        po = (h % 2) * 64
                        mc = h // 2
                        qsl = slice(qc * 512, (qc + 1) * 512)
                        s_ps = mmps.tile([P, 512], f32, tag="S")
                        # DoubleRow slots: (k_hi, q) + (k_lo, q-dup)
                        nc.tensor.matmul(
                            s_ps,
                            k8[po:po + 64, mc, :, kt * P:(kt + 1) * P],
                            q8[po:po + 64, mc, :, qsl],
                            start=True, stop=True,
                            perf_mode=DR,
                        )
                        w = 0
                        diag = kt >= 2 and kt // 4 == qc
                        if diag:
                            w = kt * P - qc * 512
                            if w > 0:
                                nc.gpsimd.memset(e_t[:, kt, 0:w], 0.0)
                        bias = cb_sb[:, kt:kt + 1] if kt < 2 else cb_sb[:, 2:3]
                        nc.scalar.activation(
                            e_t[:, kt, w:512], s_ps[:, w:512], AF.Exp,
                            bias=bias, scale=0.125,
                        )
                        if diag:
                            # zero the upper-triangle of the diagonal block
                            # post-exp (0/1 mask multiply on gpsimd)
                            nc.gpsimd.tensor_mul(
                                e_t[:, kt, w:w + P],
                                e_t[:, kt, w:w + P],
                                tri_sb[:],
                            )

                    def emit_sav(cur, prev):
                        """S matmuls of pair `cur` interleaved with AV
                        DoubleRow kt-pair matmuls of pair `prev` — spreads
                        PSUM slot demand and keeps exp lead ahead of AV."""
                        if cur is not None:
                            e_cur = epool.tile([P, NKC, 512], fp8, tag="E")
                            e_tiles[cur] = e_cur
                            skts = list(kts_for(cur[1]))
                        else:
                            skts = []
                        akp = (len(kts_for(prev[1])) // 2) if prev else 0
                        y_ps = None
                        if prev:
                            h, qc = prev
                            e_prev = e_tiles.pop(prev)
                            y_ps = yps.tile([D + 1, 512], f32, tag="Y")
                        for idx in range(max(len(skts), 2 * akp)):
                            if idx < len(skts):
                                emit_s_kt(cur[0], cur[1], e_cur, skts[idx])
                            if idx % 2 == 1 and idx // 2 < akp:
                                kp = idx // 2
                                nc.tensor.matmul(
                                    y_ps,
                                    v8[:, 2 * kp:2 * kp + 2, prev[0], :],
                                    e_prev[:, 2 * kp:2 * kp + 2, :],
                                    start=(kp == 0),
                                    stop=(kp == akp - 1),
                                    perf_mode=DR,
                                )
                        if prev:
                            emit_norm(prev[0], prev[1], y_ps)

                    def emit_norm(h, qc, y_ps):
                        po = (h % 2) * 64
                        mc = h // 2
                        qsl = slice(qc * 512, (qc + 1) * 512)
                        r_sb = asb.tile([D + 1, 512], f32, tag="r")
                        nc.vector.reciprocal(
                            r_sb[D:D + 1, :], y_ps[D:D + 1, :]
                        )
                        # partition_broadcast HW reads the tile's partition 0
                        # (AP partition offset ignored) — bounce row 64 -> 0
                        r0_sb = asb.tile([1, 512], f32, tag="r0")
                        nc.sync.dma_start(r0_sb[:], r_sb[D:D + 1, :])
                        bcs = asb.tile([64, 512], f32, tag="bcs")
                        nc.gpsimd.partition_broadcast(bcs, r0_sb[:])
                        if po == 0:
                            # even heads are already lane-aligned with the
                            # y8 destination: write directly, no DMA shift
                            nc.vector.tensor_mul(
                                y8[0:64, mc, qsl], y_ps[0:D, :], bcs
                            )
                        else:
                            yt = asb.tile([64, 512], fp8, tag="yt")
                            nc.vector.tensor_mul(yt, y_ps[0:D, :], bcs)
                            nc.sync.dma_start(y8[po:po + 64, mc, qsl], yt)

                    # qc=0 first: its stream is filled with the deferred
                    # V t4..7 chunks; the ACT-bound qc=1 stream then takes
                    # proj/LN2 for t0..3 plus 16 MLP1 n2=0 chunks whose raw
                    # psums park in u0 (gelu deferred past attention).
                    u0 = u0p.tile([P, 24, 512], bf16, tag="u0")
                    w1pre_t = {}

                    def mlp1_u_chunk(m):
                        w1th = w1p.tile([P, NKC, P], fp8, tag="w1h")
                        w1tl = w1p.tile([P, NKC, P], fp8, tag="w1l")
                        nc.sync.dma_start(
                            w1th[:], w1h_re[:, :, m * P:(m + 1) * P])
                        nc.sync.dma_start(
                            w1tl[:], w1l_re[:, :, m * P:(m + 1) * P])
                        if m >= 15:
                            # ring depth 7: these tiles stay resident for
                            # the post-attention n2=1 pass (no re-DMA)
                            w1pre_t[m] = (w1th, w1tl)
                        ps = mmps.tile([P, 512], f32, tag="S")
                        mm_ss_dr(
                            ps,
                            ((w1th, hT_h), (w1th, hT_l), (w1tl, hT_h)),
                            slice(0, P), slice(0, 512),
                        )
                        nc.vector.tensor_copy(u0[:, m, :], ps)

                    pairs = [(h, 0) for h in range(H)] + \
                            [(h, 1) for h in range(H)]
                    for i in range(len(pairs) + 1):
                        cur = pairs[i] if i < len(pairs) else None
                        prev = pairs[i - 1] if i > 0 else None
                        emit_sav(cur, prev)
                        if i > 0:
                            j = i - 1
                            if j < H:
                                # qc0 stream: V t4..7 (one per 2 pairs)
                                if j % 2 == 1:
                                    jj = j // 2
                                    emit_v(4 + jj // 2, jj % 2)
                            else:
                                jj = j - H
                                if jj < NT:
                                    emit_proj(jj // 2, jj % 2)
                                    if jj % 2 == 1:
                                        t = jj // 2
                                        ln_tile(h_tok, t, ln2_args,
                                                mean_acc=proj_acc.pop(t))
                                        transp_split(hT_h, hT_l, h_tok, t,
                                                     tp2)
                                else:
                                    mlp1_u_chunk(3 * (jj - NT))
                                    mlp1_u_chunk(3 * (jj - NT) + 1)
                                    mlp1_u_chunk(3 * (jj - NT) + 2)
                                    if jj == 15:
                                        # w1 for m<8's n2=1 streams in now,
                                        # while the DMA engines are idle
                                        for mi in range(8):
                                            th = w1pre.tile(
                                                [P, NKC, P], fp8,
                                                tag=f"w1pa{mi}",
                                                name=f"w1pa{mi}")
                                            tl = w1pre.tile(
                                                [P, NKC, P], fp8,
                                                tag=f"w1pb{mi}",
                                                name=f"w1pb{mi}")
                                            nc.sync.dma_start(
                                                th[:],
                                                w1h_re[:, :, mi * P:(mi + 1) * P])
                                            nc.sync.dma_start(
                                                tl[:],
                                                w1l_re[:, :, mi * P:(mi + 1) * P])
                                            w1pre_t[mi] = (th, tl)

                    for t in range(4, NT):
                        emit_proj(t, 0)
                        emit_proj(t, 1)
                        ln_tile(h_tok, t, ln2_args,
                                mean_acc=proj_acc.pop(t), act_sq=True)
                        transp_split(hT_h, hT_l, h_tok, t, tp2)

            # ---------------- MLP (fp8 DoubleRow, both operands hi+lo) ------
            # W1 pre-scaled x32 (unit std), W2 x64; psum scales undone at
            # evict (gelu scale=1/32, final stt scale=1/64).
            with contextlib.ExitStack() as mctx:
                gpool = mctx.enter_context(tc.tile_pool(name="g", bufs=1))
                gscr = mctx.enter_context(tc.tile_pool(name="gscr", bufs=3))
                g_h = gpool.tile([P, NM, T], fp8, tag="gh")
                # only m>=16 keeps a lo residual (hi+lo split); m<16 is
                # native e4m3 straight from ACT gelu (no cast/sub/lo-term)
                g_l = gpool.tile([P, 16, T], fp8, tag="gl")

                def mm_ss(ps, wh, wl, xh, xl, npair=NKC // 2):
                    """12 DoubleRow matmuls: hh, lh, hl over 4 kc-pairs."""
                    first = True
                    for wa, xa in ((wh, xh), (wh, xl), (wl, xh)):
                        for kp in range(npair):
                            nc.tensor.matmul(
                                ps,
                                wa[:, 2 * kp:2 * kp + 2, :],
                                xa[:, 2 * kp:2 * kp + 2, :],
                                start=first,
                                stop=(wa is wl) and (kp == npair - 1),
                                perf_mode=DR,
                            )
                            first = False

                def g_evict(m, qsl, src, b1c):
                    gsl_h = g_h[:, m, qsl]
                    if m < 16:
                        # native e4m3: one direct ACT gelu, no residual
                        if b1c is not None:
                            nc.scalar.activation(gsl_h, src, AF.Gelu,
                                                 bias=b1c, scale=1.0 / 32)
                        else:
                            nc.scalar.activation(gsl_h, src, AF.Gelu,
                                                 scale=1.0 / 32)
                        return
                    # hi+lo: gelu to bf16 scratch; hi cast on Pool, lo on DVE
                    gs = gscr.tile([P, 512], bf16, tag="gs")
                    if b1c is not None:
                        nc.scalar.activation(gs, src, AF.Gelu,
                                             bias=b1c, scale=1.0 / 32)
                    else:
                        nc.scalar.activation(gs, src, AF.Gelu,
                                             scale=1.0 / 32)
                    nc.gpsimd.tensor_copy(gsl_h, gs)
                    nc.vector.tensor_sub(g_l[:, m - 16, qsl], gs, gsl_h)

                for m in list(range(15, 24)) + list(range(15)) + \
                        list(range(24, NM)):
                    if m in w1pre_t:
                        w1th, w1tl = w1pre_t[m]
                    else:
                        w1th = w1p.tile([P, NKC, P], fp8, tag="w1h")
                        w1tl = w1p.tile([P, NKC, P], fp8, tag="w1l")
                        nc.sync.dma_start(
                            w1th[:], w1h_re[:, :, m * P:(m + 1) * P])
                        nc.sync.dma_start(
                            w1tl[:], w1l_re[:, :, m * P:(m + 1) * P])
                    b1c = opt_sb["b1"][:, m:m + 1] if b1_bias else None
                    for n2 in (1,) if m < 24 else (1, 0):
                        ps = mmps.tile([P, 512], f32, tag="S")
                        qsl = slice(n2 * 512, (n2 + 1) * 512)
                        mm_ss(ps, w1th, w1tl,
                              hT_h[:, :, qsl], hT_l[:, :, qsl])
                        g_evict(m, qsl, ps, b1c)
                    if m < 24:
                        # n2=0 raw psum was parked in u0 during attention
                        g_evict(m, slice(0, 512), u0[:, m, :], b1c)

                with (
                    tc.tile_pool(name="w2p", bufs=2) as w2p,
                    tc.tile_pool(name="outp", bufs=4) as outp,
                ):
                    for n4 in range(4):
                        nsl = slice(n4 * 256, (n4 + 1) * 256)
                        w2th = w2p.tile([P, NM, 256], fp8, tag="w2h")
                        w2tl = w2p.tile([P, NM, 256], fp8, tag="w2l")
                        nc.sync.dma_start(w2th[:], w2h_re[:, :, nsl])
                        nc.sync.dma_start(w2tl[:], w2l_re[:, :, nsl])
                        for t in range(NT):
                            ps = mmps.tile([P, 256], f32, tag="S")
                            tsl = slice(t * P, (t + 1) * P)
                            first = True
                            for ga, wa in ((g_h, w2th), (g_h, w2tl)):
                                for kp in range(NM // 2):
                                    nc.tensor.matmul(
                                        ps,
                                        ga[:, 2 * kp:2 * kp + 2, tsl],
                                        wa[:, 2 * kp:2 * kp + 2, :],
                                        start=first, stop=False,
                                        perf_mode=DR,
                                    )
                                    first = False
                            for kp in range(8, NM // 2):
                                nc.tensor.matmul(
                                    ps,
                                    g_l[:, 2 * (kp - 8):2 * (kp - 8) + 2,
                                        tsl],
                                    w2th[:, 2 * kp:2 * kp + 2, :],
                                    start=False,
                                    stop=(kp == NM // 2 - 1)
                                    and not b2_bias,
                                    perf_mode=DR,
                                )
                            if b2_bias:
                                nc.tensor.matmul(
                                    ps, ones_b[:], opt_sb["b2"][:, nsl],
                                    start=False, stop=True,
                                )
                            oc = outp.tile([P, 256], bf16, tag="oc")
                            nc.vector.scalar_tensor_tensor(
                                oc, ps, 1.0 / 64, x_sb[:, t, nsl],
                                op0=OP.mult, op1=OP.add,
                            )
                            nc.sync.dma_start(out_re[:, t, nsl], oc)

    nc.compile()
    return nc


def _host_aux(cond_mask):
    """Build per-batch cond bias [P, 2] and shared tri [P, 640] / identity."""
    counts = np.asarray(cond_mask).sum(axis=-1).astype(np.int64)  # [B]
    cbias = []
    for b in range(B):
        vec = np.full(COND_LEN, -ESHIFT, np.float32)
        vec[counts[b]:] = NEG
        cb = np.empty((P, 3), np.float32)
        cb[:, 0:2] = vec.reshape(2, P).T
        cb[:, 2] = -ESHIFT
        cbias.append(cb)
    kk = np.arange(P)[:, None]
    qq = np.arange(P)[None, :]
    tri = (qq >= kk).astype(E4M3)
    ident = np.eye(P, dtype=BF16)
    return cbias, tri, ident


def kernel(**inputs):
    from concourse.bass_utils import run_bass_kernel_spmd

    x = np.asarray(inputs["x"], np.float32)
    assert x.shape == (B, T, C)
    assert int(inputs["cond_len"]) == COND_LEN
    assert int(inputs["token_len"]) == TOKEN_LEN

    f32 = np.float32
    Wq, Wk, Wv, Wp = (np.asarray(inputs[k], f32) for k in ("Wq", "Wk", "Wv", "Wp"))
    W1, W2 = np.asarray(inputs["W1"], f32), np.asarray(inputs["W2"], f32)
    bq, bk, bv, bp = (np.asarray(inputs[k], f32) for k in ("bq", "bk", "bv", "bp"))
    b1, b2 = np.asarray(inputs["b1"], f32), np.asarray(inputs["b2"], f32)
    g1, o1 = np.asarray(inputs["ln1_g"], f32), np.asarray(inputs["ln1_b"], f32)
    g2, o2 = np.asarray(inputs["ln2_g"], f32), np.asarray(inputs["ln2_b"], f32)

    flags = (
        bool(bq.any() or bk.any()),
        bool(bv.any()),
        bool(bp.any()),
        bool(b1.any()),
        bool(b2.any()),
        bool((g1 != 1).any() or o1.any()),
        bool((g2 != 1).any() or o2.any()),
    )
    if flags not in _BUILD_CACHE:
        _BUILD_CACHE[flags] = _build(flags)
    nc = _BUILD_CACHE[flags]
    qk_bias, v_bias, p_bias, b1_bias, b2_bias, ln1_aff, ln2_aff = flags

    cbias, tri, ident = _host_aux(inputs["cond_mask"])
    w1h, w1l = _split8(W1, 32.0)
    w2h, w2l = _split8(W2, 64.0)
    shared = {
        "w1h": w1h, "w1l": w1l, "w2h": w2h, "w2l": w2l,
        "tri": tri, "ident": ident,
    }
    for wn, W in (("wq", Wq), ("wk", Wk), ("wv", Wv), ("wp", Wp)):
        hi, lo = _split8(W, 32.0)
        shared[wn + "h"], shared[wn + "l"] = hi, lo
    if qk_bias:
        shared["bq"] = np.ascontiguousarray(bq.reshape(NKC, P).T)
        shared["bk"] = np.ascontiguousarray(bk.reshape(NKC, P).T)
    if v_bias:
        shared["bv"] = (bv * 32.0).reshape(1, C).astype(BF16)
    if p_bias:
        shared["bp"] = (bp * 32.0).reshape(1, C).astype(BF16)
    if b1_bias:
        shared["b1"] = np.ascontiguousarray(b1.reshape(NM, P).T)
    if b2_bias:
        shared["b2"] = (b2 * 64.0).reshape(1, C).astype(BF16)
    if ln1_aff:
        shared["g1"] = np.broadcast_to(g1, (P, C)).copy()
        shared["o1"] = np.broadcast_to(o1, (P, C)).copy()
    if ln2_aff:
        shared["g2"] = np.broadcast_to(g2, (P, C)).copy()
        shared["o2"] = np.broadcast_to(o2, (P, C)).copy()

    in_maps = [dict(shared, x=x[b].astype(BF16), cbias=cbias[b]) for b in range(B)]
    try:
        res = run_bass_kernel_spmd(nc, in_maps, list(range(B)),
                                   trace=kernel._trace)
    except ModuleNotFoundError:
        # ntff profiling hook unavailable in this container; run untraced
        res = run_bass_kernel_spmd(nc, in_maps, list(range(B)), trace=False)
    kernel._last_results = res
    out = np.stack([res.results[b]["out"] for b in range(B)], axis=0)
    return out.astype(np.float32)


kernel._trace = False
kernel._last_results = None



# revision 75
# speedup vs baseline: 1.0089x; 1.0089x over previous
"""Trainium2 Bass kernel for nn_Block_84155589198355 (dense transformer block).

Strategy: pure data parallelism — B=8 batch elements over 8 NeuronCores, one
full transformer block per core (no collectives). Heavy matmuls run in
fp8(e4m3) DoubleRow perf mode (two 128-deep contraction slots per
instruction, 0.5 PE cycles/row = 4x bf16 throughput), with precision managed
per stage against the 2e-2 rel-err budget:

  - residual stream x kept in bf16 (host-cast), stats/psum accumulation fp32
  - weights pre-scaled by 32 (W2 by 64) to unit std and split host-side into
    e4m3 (hi, lo) pairs; psum scale undone at evict (gelu scale=1/32 etc.)
  - Q/K/V/proj: both operands native e4m3 (kc-paired DoubleRow)
  - MLP1/MLP2: both operands hi+lo ("ss": hh+lh+hl terms) — hT and gelu
    output split on-chip (gelu -> bf16 scratch; hi cast on Pool, lo on DVE)
  - S^T = K^T q: k split into (k_hi, k_lo) slots vs q duplicated across both
    slots (only stage needing duplication; dup copy on Pool)
  - AV: E (exp, shifted by ESHIFT so e4m3 never overflows; shift cancels in
    softmax) and V native e4m3, kt-paired DoubleRow; ones column in V gives
    the softmax denominator; causal triangle applied post-exp as a 0/1
    mask-multiply on Pool (gpsimd)

Schedule: qc0 attention first (V t4..7 deferred into its exp-bound stream),
then qc1 with proj/LN2 for t0..3 plus 24 MLP1 n2=0 chunks (raw psums parked
in a bf16 u0 scratch, gelu deferred past attention) as PE filler; w1 weights
for the first post-attention MLP1 iterations prefetch while the DMA engines
are idle late in attention. g is hi+lo split only for the second half of the
FF dimension (first half native e4m3 straight from ACT gelu — no cast/sub
and no lo-term in MLP2), trading ~0.4e-2 rel err for ~25us. LN mean for LN2
comes free from the proj-evict accumulator; rsqrt via bit-trick Newton on
DVE; output written bf16 (host upcasts).
"""

import sys

if "/opt/trn_rl_repo" not in sys.path:
    sys.path.insert(0, "/opt/trn_rl_repo")

import numpy as np
import ml_dtypes

B, T, C, H = 8, 1024, 1024, 16
D = C // H
FF = 4 * C
P = 128
NT = T // P      # 8 token tiles
NKC = C // P     # 8 contraction chunks over C
NM = FF // P     # 32 chunks over FF
COND_LEN = 256
TOKEN_LEN = 768
NEG = -1.0e9
EPS = 1e-5
ESHIFT = 3.0  # logit shift so exp output fits e4m3 (cancels in softmax)
BF16 = ml_dtypes.bfloat16
E4M3 = ml_dtypes.float8_e4m3


def _q8(x):
    return x.astype(E4M3)


def _split8(x, scale):
    """Return (hi, lo) e4m3 pair with hi at `scale`*x; lo at same scale."""
    xs = x * scale
    hi = _q8(xs)
    lo = _q8(xs - hi.astype(np.float32))
    return hi, lo

_BUILD_CACHE = {}


def _build(flags):
    """Build and compile the per-core Bass program. flags is a tuple of bools:
    (qk_bias, v_bias, p_bias, b1_bias, b2_bias, ln1_aff, ln2_aff)."""
    import concourse.bass as bass
    from concourse import bacc, tile, mybir

    qk_bias, v_bias, p_bias, b1_bias, b2_bias, ln1_aff, ln2_aff = flags
    f32 = mybir.dt.float32
    i32 = mybir.dt.int32
    bf16 = mybir.dt.bfloat16
    AF = mybir.ActivationFunctionType
    OP = mybir.AluOpType
    AX = mybir.AxisListType

    nc = bacc.Bacc("TRN2", target_bir_lowering=False, debug=False)

    fp8 = mybir.dt.float8e4
    DR = mybir.MatmulPerfMode.DoubleRow
    x_d = nc.dram_tensor("x", [T, C], bf16, kind="ExternalInput")
    qkvp_d = {}
    for wn in ("wq", "wk", "wv", "wp"):
        for hl in ("h", "l"):
            qkvp_d[wn + hl] = nc.dram_tensor(
                wn + hl, [C, C], fp8, kind="ExternalInput"
            )
    w1h_d = nc.dram_tensor("w1h", [C, FF], fp8, kind="ExternalInput")
    w1l_d = nc.dram_tensor("w1l", [C, FF], fp8, kind="ExternalInput")
    w2h_d = nc.dram_tensor("w2h", [FF, C], fp8, kind="ExternalInput")
    w2l_d = nc.dram_tensor("w2l", [FF, C], fp8, kind="ExternalInput")
    cb_d = nc.dram_tensor("cbias", [P, 3], f32, kind="ExternalInput")
    tri_d = nc.dram_tensor("tri", [P, P], fp8, kind="ExternalInput")
    id_d = nc.dram_tensor("ident", [P, P], bf16, kind="ExternalInput")
    out_d = nc.dram_tensor("out", [T, C], bf16, kind="ExternalOutput")

    opt_d = {}
    if qk_bias:
        opt_d["bq"] = nc.dram_tensor("bq", [P, NKC], f32, kind="ExternalInput")
        opt_d["bk"] = nc.dram_tensor("bk", [P, NKC], f32, kind="ExternalInput")
    if v_bias:
        opt_d["bv"] = nc.dram_tensor("bv", [1, C], bf16, kind="ExternalInput")
    if p_bias:
        opt_d["bp"] = nc.dram_tensor("bp", [1, C], bf16, kind="ExternalInput")
    if b1_bias:
        opt_d["b1"] = nc.dram_tensor("b1", [P, NM], f32, kind="ExternalInput")
    if b2_bias:
        opt_d["b2"] = nc.dram_tensor("b2", [1, C], bf16, kind="ExternalInput")
    if ln1_aff:
        opt_d["g1"] = nc.dram_tensor("g1", [P, C], f32, kind="ExternalInput")
        opt_d["o1"] = nc.dram_tensor("o1", [P, C], f32, kind="ExternalInput")
    if ln2_aff:
        opt_d["g2"] = nc.dram_tensor("g2", [P, C], f32, kind="ExternalInput")
        opt_d["o2"] = nc.dram_tensor("o2", [P, C], f32, kind="ExternalInput")

    x_re = x_d.ap().rearrange("(t p) c -> p t c", p=P)
    out_re = out_d.ap().rearrange("(t p) c -> p t c", p=P)
    qkvp_re = {
        nm: d.ap().rearrange("(k p) m -> p k m", p=P)
        for nm, d in qkvp_d.items()
    }
    w1h_re = w1h_d.ap().rearrange("(k p) m -> p k m", p=P)
    w1l_re = w1l_d.ap().rearrange("(k p) m -> p k m", p=P)
    w2h_re = w2h_d.ap().rearrange("(k p) m -> p k m", p=P)
    w2l_re = w2l_d.ap().rearrange("(k p) m -> p k m", p=P)

    def kts_for(qc):
        # visible k-tiles for q-chunk qc (512-wide chunks)
        return range(4) if qc == 0 else range(8)

    with tile.TileContext(nc) as tc:
        import contextlib

        with contextlib.ExitStack() as ctx:
            cpool = ctx.enter_context(tc.tile_pool(name="const", bufs=1))
            xpool = ctx.enter_context(tc.tile_pool(name="xres", bufs=1))
            apool = ctx.enter_context(tc.tile_pool(name="act", bufs=1))
            spool = ctx.enter_context(tc.tile_pool(name="small", bufs=8))
            sqpool = ctx.enter_context(tc.tile_pool(name="sqscr", bufs=1))
            # one shared [128,512] fp32 psum tag for QKV / S^T / proj / MLP2 —
            # avoids pool-boundary serialization between phases
            mmps = ctx.enter_context(
                tc.tile_pool(name="mm512", bufs=5, space="PSUM")
            )
            # w1 stream pool lives at top level so its first DMAs aren't
            # gated on the attention-phase pools releasing SBUF
            w1p = ctx.enter_context(tc.tile_pool(name="w1p", bufs=9))
            w1pre = ctx.enter_context(tc.tile_pool(name="w1pre", bufs=1))
            u0p = ctx.enter_context(tc.tile_pool(name="u0", bufs=1))

            tri_sb = cpool.tile([P, P], fp8, tag="tri")
            nc.sync.dma_start(tri_sb[:], tri_d[:])
            id_sb = cpool.tile([P, P], bf16, tag="ident")
            nc.sync.dma_start(id_sb[:], id_d[:])
            cb_sb = cpool.tile([P, 3], f32, tag="cbias")
            nc.sync.dma_start(cb_sb[:], cb_d[:])
            magic_sb = cpool.tile([P, 1], i32, tag="magic")
            nc.vector.memset(magic_sb[:], 0x5F3759DF)
            need_ones_b = v_bias or p_bias or b2_bias
            if need_ones_b:
                ones_b = cpool.tile([1, P], bf16, tag="onesb")
                nc.gpsimd.memset(ones_b[:], 1.0)
            opt_sb = {}
            for nm, dd in opt_d.items():
                shp = list(dd.shape)
                dt_ = dd.dtype
                opt_sb[nm] = cpool.tile(shp, dt_, tag=nm)
                nc.sync.dma_start(opt_sb[nm][:], dd[:])

            x_sb = xpool.tile([P, NT, C], bf16, tag="x")
            for t in range(4):
                nc.sync.dma_start(x_sb[:, t, :], x_re[:, t, :])

            # ---------------- LayerNorm (token-major) + transpose ----------
            def ln_tile(dst_tok, t, affine, act_mean=False, mean_acc=None,
                        norm_act=False, act_sq=False):
                    xr = x_sb[:, t, :]
                    mu = spool.tile([P, 1], f32, tag="mu")
                    if mean_acc is not None:
                        # row-sums already accumulated by the residual-evict
                        nc.vector.tensor_add(
                            mu, mean_acc[:, 0:1], mean_acc[:, 1:2]
                        )
                        nc.vector.tensor_scalar_mul(mu, mu, 1.0 / C)
                    elif act_mean:
                        # mean via ACT Copy+accum (frees DVE on the startup
                        # critical path; Copy shares exp's LUT set)
                        cs = sqpool.tile([P, C], bf16, tag="sq")
                        nc.scalar.activation(cs, xr, AF.Copy, accum_out=mu)
                        nc.vector.tensor_scalar_mul(mu, mu, 1.0 / C)
                    else:
                        nc.vector.tensor_reduce(mu, xr, axis=AX.X, op=OP.add)
                        nc.vector.tensor_scalar_mul(mu, mu, 1.0 / C)
                    sq = sqpool.tile([P, C], bf16, tag="sq")
                    ss = spool.tile([P, 1], f32, tag="ss")
                    if mean_acc is not None and not act_sq:
                        # qc1-window LN: keep ACT free for exp — square on DVE
                        nc.vector.scalar_tensor_tensor(
                            sq, xr, 1.0, xr, op0=OP.mult, op1=OP.mult,
                            accum_out=ss,
                        )
                    else:
                        nc.scalar.activation(sq, xr, AF.Square, accum_out=ss)
                    var = spool.tile([P, 1], f32, tag="var")
                    musq = spool.tile([P, 1], f32, tag="musq")
                    nc.vector.tensor_mul(musq, mu, mu)
                    nc.vector.tensor_scalar_mul(var, ss, 1.0 / C)
                    nc.vector.tensor_sub(var, var, musq)
                    nc.vector.tensor_scalar_add(var, var, EPS)
                    # rstd = rsqrt(var) on DVE (bit-trick + 3 Newton steps):
                    # ACT Sqrt/Ln would thrash LUT-table loads against the
                    # attention exp stream (different act_func_sets)
                    rstd = spool.tile([P, 1], f32, tag="rstd")
                    ri = rstd[:].bitcast(i32)
                    nc.vector.tensor_single_scalar(
                        ri, var[:].bitcast(i32), 1, op=OP.arith_shift_right
                    )
                    nc.vector.tensor_sub(ri, magic_sb[:], ri)
                    nsq = spool.tile([P, 1], f32, tag="nsq")
                    for _ in range(2):
                        nc.vector.tensor_mul(nsq, rstd, rstd)
                        nc.vector.tensor_mul(nsq, nsq, var)
                        nc.vector.tensor_scalar(
                            nsq, nsq, -0.5, 1.5, op0=OP.mult, op1=OP.add
                        )
                        nc.vector.tensor_mul(rstd, rstd, nsq)
                    if affine is None and norm_act:
                        # xn = Identity(x*rstd + (-mu*rstd)) on ACT — used on
                        # alternate LN1 tiles to split the normalize pass
                        # across both engines (Identity shares exp's LUT set)
                        nmr = spool.tile([P, 1], f32, tag="nmr")
                        nc.vector.tensor_mul(nmr, mu, rstd)
                        nc.vector.tensor_scalar_mul(nmr, nmr, -1.0)
                        nc.scalar.activation(
                            dst_tok[:, t, :], xr, AF.Identity,
                            bias=nmr, scale=rstd,
                        )
                    elif affine is None:
                        nc.vector.tensor_scalar(
                            dst_tok[:, t, :], xr, mu, rstd,
                            op0=OP.subtract, op1=OP.mult,
                        )
                    else:
                        g_sb_, o_sb_ = affine
                        tmp = spool.tile([P, C], f32, tag="lntmp")
                        nc.vector.tensor_scalar(
                            tmp, xr, mu, rstd, op0=OP.subtract, op1=OP.mult
                        )
                        nc.vector.tensor_mul(tmp, tmp, g_sb_[:])
                        nc.vector.tensor_add(dst_tok[:, t, :], tmp, o_sb_[:])

            def transp_tile(dst_fT, src_tok, t, psum_pool, tag="tp"):
                for mc in range(NKC):
                    tp = psum_pool.tile([P, P], bf16, tag=tag,
                                        name=f"tp{t}_{mc}")
                    nc.tensor.transpose(
                        tp, src_tok[:, t, mc * P:(mc + 1) * P], id_sb[:]
                    )
                    nc.vector.tensor_copy(
                        dst_fT[:, mc, t * P:(t + 1) * P], tp
                    )

            def transp_one(dst_h, src_tok, t, psum_pool, tag="tp"):
                # batched transpose, single e4m3 evict (no lo residual)
                for half in range(2):
                    mc0 = half * 4
                    tp = psum_pool.tile([P, 512], bf16, tag=tag,
                                        name=f"t1_{t}_{half}")
                    for i in range(4):
                        nc.tensor.transpose(
                            tp[:, i * P:(i + 1) * P],
                            src_tok[:, t, (mc0 + i) * P:(mc0 + i + 1) * P],
                            id_sb[:],
                        )
                    nc.vector.tensor_copy(
                        dst_h[:, mc0:mc0 + 4, t * P:(t + 1) * P],
                        tp.rearrange("p (k q) -> p k q", q=P),
                    )

            def transp_split(dst_h, dst_l, src_tok, t, psum_pool, tag="tp"):
                # transpose 4 feature blocks into one [P,512] psum, then
                # evict as e4m3 hi + lo (lo = exact - hi)
                for half in range(2):
                    mc0 = half * 4
                    tp = psum_pool.tile([P, 512], bf16, tag=tag,
                                        name=f"tsp{t}_{half}")
                    for i in range(4):
                        nc.tensor.transpose(
                            tp[:, i * P:(i + 1) * P],
                            src_tok[:, t, (mc0 + i) * P:(mc0 + i + 1) * P],
                            id_sb[:],
                        )
                    tp3 = tp.rearrange("p (k q) -> p k q", q=P)
                    hsl = dst_h[:, mc0:mc0 + 4, t * P:(t + 1) * P]
                    nc.vector.tensor_copy(hsl, tp3)
                    nc.vector.tensor_sub(
                        dst_l[:, mc0:mc0 + 4, t * P:(t + 1) * P], tp3, hsl
                    )

            # ---------------- QKV + attention + proj -----------------------
            # q8: [*, mc, 2(dup), T]; k8: [*, mc, 2(hi|lo), T] so the S^T
            # DoubleRow pairs (k_hi,q)+(k_lo,q). v8/y8 single e4m3.
            with contextlib.ExitStack() as actx:
                qkvy = actx.enter_context(tc.tile_pool(name="qkvy", bufs=1))
                wpool = actx.enter_context(tc.tile_pool(name="wstream", bufs=2))
                # wq streams ahead of the second x half so the first QK
                # matmuls aren't DMA-gated
                wq_th = wpool.tile([P, NKC, C], fp8, tag="wh")
                nc.sync.dma_start(wq_th[:], qkvp_re["wqh"])
                for t in range(4, NT):
                    nc.sync.dma_start(x_sb[:, t, :], x_re[:, t, :])

                ln1_args = (opt_sb["g1"][:], opt_sb["o1"][:]) if ln1_aff else None
                ln2_args = (opt_sb["g2"][:], opt_sb["o2"][:]) if ln2_aff else None
                xn_tok = apool.tile([P, NT, C], bf16, tag="tok")
                xnT_h = apool.tile([P, NKC, T], fp8, tag="xTh")
                with tc.tile_pool(name="tpsum", bufs=2, space="PSUM") as tpp:
                    for t in range(NT):
                        ln_tile(xn_tok, t, ln1_args)
                        transp_one(xnT_h, xn_tok, t, tpp)

                q8 = qkvy.tile([P, NKC, 2, T], fp8, tag="q")
                k8 = qkvy.tile([P, NKC, 2, T], fp8, tag="k")
                v8 = qkvy.tile([P, NT, H, D + 1], fp8, tag="v")
                y8 = qkvy.tile([P, NKC, T], fp8, tag="y")
                nc.vector.memset(v8[:, :, :, D:D + 1], 1.0)

                def mm_ss_dr(ps, terms, qsl_m, qsl_x, stop_ok=True):
                    """DoubleRow kc-paired products: terms = [(w, x), ...]"""
                    first = True
                    for ti, (wa, xa) in enumerate(terms):
                        for kp in range(NKC // 2):
                            nc.tensor.matmul(
                                ps,
                                wa[:, 2 * kp:2 * kp + 2, qsl_m],
                                xa[:, 2 * kp:2 * kp + 2, qsl_x],
                                start=first,
                                stop=stop_ok and (ti == len(terms) - 1)
                                and (kp == NKC // 2 - 1),
                                perf_mode=DR,
                            )
                            first = False

                # Q and K (feature-major); n2-outer so the first token half's
                # xnT transposes unblock matmuls early
                for which, dst in ((0, q8), (1, k8)):
                    if which == 0:
                        wht = wq_th
                    else:
                        wht = wpool.tile([P, NKC, C], fp8, tag="wh")
                        nc.sync.dma_start(wht[:], qkvp_re["wkh"])
                    for n2 in (0, 1):
                        for m in range(NKC):
                            ps = mmps.tile([P, 512], f32, tag="S")
                            qsl = slice(n2 * 512, (n2 + 1) * 512)
                            mm_ss_dr(
                                ps,
                                ((wht, xnT_h),),
                                slice(m * P, (m + 1) * P), qsl,
                            )
                            d0 = dst[:, m, 0, qsl]
                            d1 = dst[:, m, 1, qsl]
                            if qk_bias:
                                bias_nm = "bq" if which == 0 else "bk"
                                sc = sqpool.tile([P, 512], bf16, tag="qksc")
                                nc.scalar.activation(
                                    sc, ps, AF.Identity,
                                    bias=opt_sb[bias_nm][:, m:m + 1],
                                    scale=1.0 / 32,
                                )
                                nc.vector.tensor_copy(d0, sc)
                                if which == 0:
                                    nc.gpsimd.tensor_copy(d1, d0)
                                else:
                                    nc.vector.tensor_sub(d1, sc, d0)
                            else:
                                nc.scalar.activation(d0, ps, AF.Identity,
                                                     scale=1.0 / 32)
                                if which == 0:
                                    # q duplicated across both DR slots
                                    nc.gpsimd.tensor_copy(d1, d0)
                                else:
                                    # k_lo = exact - k_hi
                                    nc.vector.scalar_tensor_tensor(
                                        d1, ps, 1.0 / 32, d0,
                                        op0=OP.mult, op1=OP.subtract,
                                    )

                # V (token-major, strided into per-head 65-wide slots).
                # n2=0 (heads 0..7) now; n2=1 groups are deferred into the
                # ACT-bound qc1 attention stream as PE filler (heads 8..15
                # aren't consumed until the 9th qc1 pair).
                wvh_sb = wpool.tile([P, NKC, C], fp8, tag="wh")
                nc.sync.dma_start(wvh_sb[:], qkvp_re["wvh"])

                def emit_v(t, n2):
                    ps = mmps.tile([P, 512], f32, tag="S")
                    nsl = slice(n2 * 512, (n2 + 1) * 512)
                    tsl = slice(t * P, (t + 1) * P)
                    first = True
                    for kp in range(NKC // 2):
                        nc.tensor.matmul(
                            ps,
                            xnT_h[:, 2 * kp:2 * kp + 2, tsl],
                            wvh_sb[:, 2 * kp:2 * kp + 2, nsl],
                            start=first,
                            stop=(kp == NKC // 2 - 1) and not v_bias,
                            perf_mode=DR,
                        )
                        first = False
                    if v_bias:
                        nc.tensor.matmul(
                            ps, ones_b[:],
                            opt_sb["bv"][:, n2 * 512:(n2 + 1) * 512],
                            start=False, stop=True,
                        )
                    nc.scalar.activation(
                        v8[:, t, n2 * 8:(n2 + 1) * 8, 0:D],
                        ps.rearrange("p (h d) -> p h d", d=D),
                        AF.Identity, scale=1.0 / 32,
                    )

                for t in range(4):
                    emit_v(t, 0)

                # ---- attention (qc0 first) with V t4..7 filling the qc0
                # stream and proj/LN2 for t0..3 filling the qc1 stream ----
                h_tok = apool.tile([P, NT, C], bf16, tag="tok")
                hT_h = apool.tile([P, NKC, T], fp8, tag="fTh")
                hT_l = apool.tile([P, NKC, T], fp8, tag="fTl")
                wph_sb = wpool.tile([P, NKC, C], fp8, tag="wh")
                nc.sync.dma_start(wph_sb[:], qkvp_re["wph"])

                def emit_proj(t, n2):
                    # proj is y8(native) @ Wp(hi+lo), mc-paired DoubleRow
                    ps = mmps.tile([P, 512], f32, tag="S")
                    nsl = slice(n2 * 512, (n2 + 1) * 512)
                    tsl = slice(t * P, (t + 1) * P)
                    first = True
                    for kp in range(NKC // 2):
                        nc.tensor.matmul(
                            ps,
                            y8[:, 2 * kp:2 * kp + 2, tsl],
                            wph_sb[:, 2 * kp:2 * kp + 2, nsl],
                            start=first,
                            stop=(kp == NKC // 2 - 1) and not p_bias,
                            perf_mode=DR,
                        )
                        first = False
                    if p_bias:
                        nc.tensor.matmul(
                            ps, ones_b[:],
                            opt_sb["bp"][:, n2 * 512:(n2 + 1) * 512],
                            start=False, stop=True,
                        )
                    xsl = x_sb[:, t, n2 * 512:(n2 + 1) * 512]
                    if t not in proj_acc:
                        proj_acc[t] = spool.tile([P, 2], f32, tag="pacc", name=f"pacc{t}")
                    # x1 = ps/32 + x, with the row-sum accumulated on the side
                    # so LN2 doesn't need its own mean-reduction pass
                    nc.vector.scalar_tensor_tensor(
                        xsl, ps, 1.0 / 32, xsl, op0=OP.mult, op1=OP.add,
                        accum_out=proj_acc[t][:, n2:n2 + 1],
                    )

                proj_acc = {}
                with (
                    tc.tile_pool(name="epool", bufs=2) as epool,
                    tc.tile_pool(name="attpy", bufs=2, space="PSUM") as yps,
                    tc.tile_pool(name="tpsum2", bufs=1, space="PSUM") as tp2,
                    tc.tile_pool(name="attsb", bufs=2) as asb,
                ):
                    e_tiles = {}

                    def emit_s_kt(h, qc, e_t, kt):
                        po = (h % 2) * 64
                        mc = h // 2
                        qsl = slice(qc * 512, (qc + 1) * 512)
                        s_ps = mmps.tile([P, 512], f32, tag="S")
                        # DoubleRow slots: (k_hi, q) + (k_lo, q-dup)
                        nc.tensor.matmul(
                            s_ps,
                            k8[po:po + 64, mc, :, kt * P:(kt + 1) * P],
                            q8[po:po + 64, mc, :, qsl],
                            start=True, stop=True,
                            perf_mode=DR,
                        )
                        w = 0
                        diag = kt >= 2 and kt // 4 == qc
                        if diag:
                            w = kt * P - qc * 512
                            if w > 0:
                                nc.gpsimd.memset(e_t[:, kt, 0:w], 0.0)
                        bias = cb_sb[:, kt:kt + 1] if kt < 2 else cb_sb[:, 2:3]
                        nc.scalar.activation(
                            e_t[:, kt, w:512], s_ps[:, w:512], AF.Exp,
                            bias=bias, scale=0.125,
                        )
                        if diag:
                            # zero the upper-triangle of the diagonal block
                            # post-exp (0/1 mask multiply on gpsimd)
                            nc.gpsimd.tensor_mul(
                                e_t[:, kt, w:w + P],
                                e_t[:, kt, w:w + P],
                                tri_sb[:],
                            )

                    def emit_sav(cur, prev):
                        """S matmuls of pair `cur` interleaved with AV
                        DoubleRow kt-pair matmuls of pair `prev` — spreads
                        PSUM slot demand and keeps exp lead ahead of AV."""
                        if cur is not None:
                            e_cur = epool.tile([P, NKC, 512], fp8, tag="E")
                            e_tiles[cur] = e_cur
                            skts = list(kts_for(cur[1]))
                        else:
                            skts = []
                        akp = (len(kts_for(prev[1])) // 2) if prev else 0
                        y_ps = None
                        if prev:
                            h, qc = prev
                            e_prev = e_tiles.pop(prev)
                            y_ps = yps.tile([D + 1, 512], f32, tag="Y")
                        for idx in range(max(len(skts), 2 * akp)):
                            if idx < len(skts):
                                emit_s_kt(cur[0], cur[1], e_cur, skts[idx])
                            if idx % 2 == 1 and idx // 2 < akp:
                                kp = idx // 2
                                nc.tensor.matmul(
                                    y_ps,
                                    v8[:, 2 * kp:2 * kp + 2, prev[0], :],
                                    e_prev[:, 2 * kp:2 * kp + 2, :],
                                    start=(kp == 0),
                                    stop=(kp == akp - 1),
                                    perf_mode=DR,
                                )
                        if prev:
                            emit_norm(prev[0], prev[1], y_ps)

                    def emit_norm(h, qc, y_ps):
                        po = (h % 2) * 64
                        mc = h // 2
                        qsl = slice(qc * 512, (qc + 1) * 512)
                        r_sb = asb.tile([D + 1, 512], f32, tag="r")
                        nc.vector.reciprocal(
                            r_sb[D:D + 1, :], y_ps[D:D + 1, :]
                        )
                        # partition_broadcast HW reads the tile's partition 0
                        # (AP partition offset ignored) — bounce row 64 -> 0
                        r0_sb = asb.tile([1, 512], f32, tag="r0")
                        nc.sync.dma_start(r0_sb[:], r_sb[D:D + 1, :])
                        bcs = asb.tile([64, 512], f32, tag="bcs")
                        nc.gpsimd.partition_broadcast(bcs, r0_sb[:])
                        if po == 0:
                            # even heads are already lane-aligned with the
                            # y8 destination: write directly, no DMA shift
                            nc.vector.tensor_mul(
                                y8[0:64, mc, qsl], y_ps[0:D, :], bcs
                            )
                        else:
                            yt = asb.tile([64, 512], fp8, tag="yt")
                            nc.vector.tensor_mul(yt, y_ps[0:D, :], bcs)
                            nc.sync.dma_start(y8[po:po + 64, mc, qsl], yt)

                    # qc=0 first: its stream is filled with the deferred
                    # V t4..7 chunks; the ACT-bound qc=1 stream then takes
                    # proj/LN2 for t0..3 plus 16 MLP1 n2=0 chunks whose raw
                    # psums park in u0 (gelu deferred past attention).
                    u0 = u0p.tile([P, 24, 512], bf16, tag="u0")
                    w1pre_t = {}

                    def mlp1_u_chunk(m):
                        w1th = w1p.tile([P, NKC, P], fp8, tag="w1h")
                        w1tl = w1p.tile([P, NKC, P], fp8, tag="w1l")
                        nc.sync.dma_start(
                            w1th[:], w1h_re[:, :, m * P:(m + 1) * P])
                        nc.sync.dma_start(
                            w1tl[:], w1l_re[:, :, m * P:(m + 1) * P])
                        if m >= 15:
                            # ring depth 7: these tiles stay resident for
                            # the post-attention n2=1 pass (no re-DMA)
                            w1pre_t[m] = (w1th, w1tl)
                        ps = mmps.tile([P, 512], f32, tag="S")
                        mm_ss_dr(
                            ps,
                            ((w1th, hT_h), (w1th, hT_l), (w1tl, hT_h)),
                            slice(0, P), slice(0, 512),
                        )
                        nc.vector.tensor_copy(u0[:, m, :], ps)

                    pairs = [(h, 0) for h in range(H)] + \
                            [(h, 1) for h in range(H)]
                    for i in range(len(pairs) + 1):
                        cur = pairs[i] if i < len(pairs) else None
                        prev = pairs[i - 1] if i > 0 else None
                        emit_sav(cur, prev)
                        if i > 0:
                            j = i - 1
                            if j < H:
                                # qc0 stream: V t0..3 n2=1 (even j, needed
                                # from pair 8) and V t4..7 (odd j)
                                if j % 2 == 0 and j < 8:
                                    emit_v(j // 2, 1)
                                elif j % 2 == 1:
                                    jj = j // 2
                                    emit_v(4 + jj // 2, jj % 2)
                            else:
                                jj = j - H
                                if jj < NT:
                                    emit_proj(jj // 2, jj % 2)
                                    if jj % 2 == 1:
                                        t = jj // 2
                                        ln_tile(h_tok, t, ln2_args,
                                                mean_acc=proj_acc.pop(t))
                                        transp_split(hT_h, hT_l, h_tok, t,
                                                     tp2)
                                else:
                                    mlp1_u_chunk(3 * (jj - NT))
                                    mlp1_u_chunk(3 * (jj - NT) + 1)
                                    mlp1_u_chunk(3 * (jj - NT) + 2)
                                    if jj == 15:
                                        # w1 for m<8's n2=1 streams in now,
                                        # while the DMA engines are idle
                                        for mi in range(8):
                                            th = w1pre.tile(
                                                [P, NKC, P], fp8,
                                                tag=f"w1pa{mi}",
                                                name=f"w1pa{mi}")
                                            tl = w1pre.tile(
                                                [P, NKC, P], fp8,
                                                tag=f"w1pb{mi}",
                                                name=f"w1pb{mi}")
                                            nc.sync.dma_start(
                                                th[:],
                                                w1h_re[:, :, mi * P:(mi + 1) * P])
                                            nc.sync.dma_start(
                                                tl[:],
                                                w1l_re[:, :, mi * P:(mi + 1) * P])
                                            w1pre_t[mi] = (th, tl)

                    for t in range(4, NT):
                        emit_proj(t, 0)
                        emit_proj(t, 1)
                        ln_tile(h_tok, t, ln2_args,
                                mean_acc=proj_acc.pop(t), act_sq=True)
                        transp_split(hT_h, hT_l, h_tok, t, tp2)

            # ---------------- MLP (fp8 DoubleRow, both operands hi+lo) ------
            # W1 pre-scaled x32 (unit std), W2 x64; psum scales undone at
            # evict (gelu scale=1/32, final stt scale=1/64).
            with contextlib.ExitStack() as mctx:
                gpool = mctx.enter_context(tc.tile_pool(name="g", bufs=1))
                gscr = mctx.enter_context(tc.tile_pool(name="gscr", bufs=3))
                g_h = gpool.tile([P, NM, T], fp8, tag="gh")
                # only m>=16 keeps a lo residual (hi+lo split); m<16 is
                # native e4m3 straight from ACT gelu (no cast/sub/lo-term)
                g_l = gpool.tile([P, 16, T], fp8, tag="gl")

                def mm_ss(ps, wh, wl, xh, xl, npair=NKC // 2):
                    """12 DoubleRow matmuls: hh, lh, hl over 4 kc-pairs."""
                    first = True
                    for wa, xa in ((wh, xh), (wh, xl), (wl, xh)):
                        for kp in range(npair):
                            nc.tensor.matmul(
                                ps,
                                wa[:, 2 * kp:2 * kp + 2, :],
                                xa[:, 2 * kp:2 * kp + 2, :],
                                start=first,
                                stop=(wa is wl) and (kp == npair - 1),
                                perf_mode=DR,
                            )
                            first = False

                def g_evict(m, qsl, src, b1c):
                    gsl_h = g_h[:, m, qsl]
                    if m < 16:
                        # native e4m3: one direct ACT gelu, no residual
                        if b1c is not None:
                            nc.scalar.activation(gsl_h, src, AF.Gelu,
                                                 bias=b1c, scale=1.0 / 32)
                        else:
                            nc.scalar.activation(gsl_h, src, AF.Gelu,
                                                 scale=1.0 / 32)
                        return
                    # hi+lo: gelu to bf16 scratch; hi cast on Pool, lo on DVE
                    gs = gscr.tile([P, 512], bf16, tag="gs")
                    if b1c is not None:
                        nc.scalar.activation(gs, src, AF.Gelu,
                                             bias=b1c, scale=1.0 / 32)
                    else:
                        nc.scalar.activation(gs, src, AF.Gelu,
                                             scale=1.0 / 32)
                    nc.gpsimd.tensor_copy(gsl_h, gs)
                    nc.vector.tensor_sub(g_l[:, m - 16, qsl], gs, gsl_h)

                for m in list(range(15, 24)) + list(range(15)) + \
                        list(range(24, NM)):
                    if m in w1pre_t:
                        w1th, w1tl = w1pre_t[m]
                    else:
                        w1th = w1p.tile([P, NKC, P], fp8, tag="w1h")
                        w1tl = w1p.tile([P, NKC, P], fp8, tag="w1l")
                        nc.sync.dma_start(
                            w1th[:], w1h_re[:, :, m * P:(m + 1) * P])
                        nc.sync.dma_start(
                            w1tl[:], w1l_re[:, :, m * P:(m + 1) * P])
                    b1c = opt_sb["b1"][:, m:m + 1] if b1_bias else None
                    for n2 in (1,) if m < 24 else (1, 0):
                        ps = mmps.tile([P, 512], f32, tag="S")
                        qsl = slice(n2 * 512, (n2 + 1) * 512)
                        mm_ss(ps, w1th, w1tl,
                              hT_h[:, :, qsl], hT_l[:, :, qsl])
                        g_evict(m, qsl, ps, b1c)
                    if m < 24:
                        # n2=0 raw psum was parked in u0 during attention
                        g_evict(m, slice(0, 512), u0[:, m, :], b1c)

                with (
                    tc.tile_pool(name="w2p", bufs=2) as w2p,
                    tc.tile_pool(name="outp", bufs=4) as outp,
                ):
                    for n4 in range(4):
                        nsl = slice(n4 * 256, (n4 + 1) * 256)
                        w2th = w2p.tile([P, NM, 256], fp8, tag="w2h")
                        w2tl = w2p.tile([P, NM, 256], fp8, tag="w2l")
                        nc.sync.dma_start(w2th[:], w2h_re[:, :, nsl])
                        nc.sync.dma_start(w2tl[:], w2l_re[:, :, nsl])
                        for t in range(NT):
                            ps = mmps.tile([P, 256], f32, tag="S")
                            tsl = slice(t * P, (t + 1) * P)
                            first = True
                            for ga, wa in ((g_h, w2th), (g_h, w2tl)):
                                for kp in range(NM // 2):
                                    nc.tensor.matmul(
                                        ps,
                                        ga[:, 2 * kp:2 * kp + 2, tsl],
                                        wa[:, 2 * kp:2 * kp + 2, :],
                                        start=first, stop=False,
                                        perf_mode=DR,
                                    )
                                    first = False
                            for kp in range(8, NM // 2):
                                nc.tensor.matmul(
                                    ps,
                                    g_l[:, 2 * (kp - 8):2 * (kp - 8) + 2,
                                        tsl],
                                    w2th[:, 2 * kp:2 * kp + 2, :],
                                    start=False,
                                    stop=(kp == NM // 2 - 1)
                                    and not b2_bias,
                                    perf_mode=DR,
                                )
                            if b2_bias:
                                nc.tensor.matmul(
                                    ps, ones_b[:], opt_sb["b2"][:, nsl],
                                    start=False, stop=True,
                                )
                            oc = outp.tile([P, 256], bf16, tag="oc")
                            nc.vector.scalar_tensor_tensor(
                                oc, ps, 1.0 / 64, x_sb[:, t, nsl],
                                op0=OP.mult, op1=OP.add,
                            )
                            nc.sync.dma_start(out_re[:, t, nsl], oc)

    nc.compile()
    return nc


def _host_aux(cond_mask):
    """Build per-batch cond bias [P, 2] and shared tri [P, 640] / identity."""
    counts = np.asarray(cond_mask).sum(axis=-1).astype(np.int64)  # [B]
    cbias = []
    for b in range(B):
        vec = np.full(COND_LEN, -ESHIFT, np.float32)
        vec[counts[b]:] = NEG
        cb = np.empty((P, 3), np.float32)
        cb[:, 0:2] = vec.reshape(2, P).T
        cb[:, 2] = -ESHIFT
        cbias.append(cb)
    kk = np.arange(P)[:, None]
    qq = np.arange(P)[None, :]
    tri = (qq >= kk).astype(E4M3)
    ident = np.eye(P, dtype=BF16)
    return cbias, tri, ident


def kernel(**inputs):
    from concourse.bass_utils import run_bass_kernel_spmd

    x = np.asarray(inputs["x"], np.float32)
    assert x.shape == (B, T, C)
    assert int(inputs["cond_len"]) == COND_LEN
    assert int(inputs["token_len"]) == TOKEN_LEN

    f32 = np.float32
    Wq, Wk, Wv, Wp = (np.asarray(inputs[k], f32) for k in ("Wq", "Wk", "Wv", "Wp"))
    W1, W2 = np.asarray(inputs["W1"], f32), np.asarray(inputs["W2"], f32)
    bq, bk, bv, bp = (np.asarray(inputs[k], f32) for k in ("bq", "bk", "bv", "bp"))
    b1, b2 = np.asarray(inputs["b1"], f32), np.asarray(inputs["b2"], f32)
    g1, o1 = np.asarray(inputs["ln1_g"], f32), np.asarray(inputs["ln1_b"], f32)
    g2, o2 = np.asarray(inputs["ln2_g"], f32), np.asarray(inputs["ln2_b"], f32)

    flags = (
        bool(bq.any() or bk.any()),
        bool(bv.any()),
        bool(bp.any()),
        bool(b1.any()),
        bool(b2.any()),
        bool((g1 != 1).any() or o1.any()),
        bool((g2 != 1).any() or o2.any()),
    )
    if flags not in _BUILD_CACHE:
        _BUILD_CACHE[flags] = _build(flags)
    nc = _BUILD_CACHE[flags]
    qk_bias, v_bias, p_bias, b1_bias, b2_bias, ln1_aff, ln2_aff = flags

    cbias, tri, ident = _host_aux(inputs["cond_mask"])
    w1h, w1l = _split8(W1, 32.0)
    w2h, w2l = _split8(W2, 64.0)
    shared = {
        "w1h": w1h, "w1l": w1l, "w2h": w2h, "w2l": w2l,
        "tri": tri, "ident": ident,
    }
    for wn, W in (("wq", Wq), ("wk", Wk), ("wv", Wv), ("wp", Wp)):
        hi, lo = _split8(W, 32.0)
        shared[wn + "h"], shared[wn + "l"] = hi, lo
    if qk_bias:
        shared["bq"] = np.ascontiguousarray(bq.reshape(NKC, P).T)
        shared["bk"] = np.ascontiguousarray(bk.reshape(NKC, P).T)
    if v_bias:
        shared["bv"] = (bv * 32.0).reshape(1, C).astype(BF16)
    if p_bias:
        shared["bp"] = (bp * 32.0).reshape(1, C).astype(BF16)
    if b1_bias:
        shared["b1"] = np.ascontiguousarray(b1.reshape(NM, P).T)
    if b2_bias:
        shared["b2"] = (b2 * 64.0).reshape(1, C).astype(BF16)
    if ln1_aff:
        shared["g1"] = np.broadcast_to(g1, (P, C)).copy()
        shared["o1"] = np.broadcast_to(o1, (P, C)).copy()
    if ln2_aff:
        shared["g2"] = np.broadcast_to(g2, (P, C)).copy()
        shared["o2"] = np.broadcast_to(o2, (P, C)).copy()

    in_maps = [dict(shared, x=x[b].astype(BF16), cbias=cbias[b]) for b in range(B)]
    try:
        res = run_bass_kernel_spmd(nc, in_maps, list(range(B)),
                                   trace=kernel._trace)
    except ModuleNotFoundError:
        # ntff profiling hook unavailable in this container; run untraced
        res = run_bass_kernel_spmd(nc, in_maps, list(range(B)), trace=False)
    kernel._last_results = res
    out = np.stack([res.results[b]["out"] for b in range(B)], axis=0)
    return out.astype(np.float32)


kernel._trace = False
kernel._last_results = None



# revision 77
# speedup vs baseline: 1.0227x; 1.0136x over previous
"""Trainium2 Bass kernel for nn_Block_84155589198355 (dense transformer block).

Strategy: pure data parallelism — B=8 batch elements over 8 NeuronCores, one
full transformer block per core (no collectives). Heavy matmuls run in
fp8(e4m3) DoubleRow perf mode (two 128-deep contraction slots per
instruction, 0.5 PE cycles/row = 4x bf16 throughput), with precision managed
per stage against the 2e-2 rel-err budget:

  - residual stream x kept in bf16 (host-cast), stats/psum accumulation fp32
  - weights pre-scaled by 32 (W2 by 64) to unit std and split host-side into
    e4m3 (hi, lo) pairs; psum scale undone at evict (gelu scale=1/32 etc.)
  - Q/K/V/proj: both operands native e4m3 (kc-paired DoubleRow)
  - MLP1/MLP2: both operands hi+lo ("ss": hh+lh+hl terms) — hT and gelu
    output split on-chip (gelu -> bf16 scratch; hi cast on Pool, lo on DVE)
  - S^T = K^T q: k split into (k_hi, k_lo) slots vs q duplicated across both
    slots (only stage needing duplication; dup copy on Pool)
  - AV: E (exp, shifted by ESHIFT so e4m3 never overflows; shift cancels in
    softmax) and V native e4m3, kt-paired DoubleRow; ones column in V gives
    the softmax denominator; causal triangle applied post-exp as a 0/1
    mask-multiply on Pool (gpsimd)

Schedule: qc0 attention first (V t4..7 and the t0..3 head-half 8..15
chunks deferred into its exp-bound stream),
then qc1 with proj/LN2 for t0..3 plus 24 MLP1 n2=0 chunks (raw psums parked
in a bf16 u0 scratch, gelu deferred past attention) as PE filler; w1 weights
for the first post-attention MLP1 iterations prefetch while the DMA engines
are idle late in attention. g is hi+lo split only for the second half of the
FF dimension (first half native e4m3 straight from ACT gelu — no cast/sub
and no lo-term in MLP2), trading ~0.4e-2 rel err for ~25us. LN mean for LN2
comes free from the proj-evict accumulator; rsqrt via bit-trick Newton on
DVE; output written bf16 (host upcasts).
"""

import sys

if "/opt/trn_rl_repo" not in sys.path:
    sys.path.insert(0, "/opt/trn_rl_repo")

import numpy as np
import ml_dtypes

B, T, C, H = 8, 1024, 1024, 16
D = C // H
FF = 4 * C
P = 128
NT = T // P      # 8 token tiles
NKC = C // P     # 8 contraction chunks over C
NM = FF // P     # 32 chunks over FF
COND_LEN = 256
TOKEN_LEN = 768
NEG = -1.0e9
EPS = 1e-5
ESHIFT = 3.0  # logit shift so exp output fits e4m3 (cancels in softmax)
BF16 = ml_dtypes.bfloat16
E4M3 = ml_dtypes.float8_e4m3


def _q8(x):
    return x.astype(E4M3)


def _split8(x, scale):
    """Return (hi, lo) e4m3 pair with hi at `scale`*x; lo at same scale."""
    xs = x * scale
    hi = _q8(xs)
    lo = _q8(xs - hi.astype(np.float32))
    return hi, lo

_BUILD_CACHE = {}


def _build(flags):
    """Build and compile the per-core Bass program. flags is a tuple of bools:
    (qk_bias, v_bias, p_bias, b1_bias, b2_bias, ln1_aff, ln2_aff)."""
    import concourse.bass as bass
    from concourse import bacc, tile, mybir

    qk_bias, v_bias, p_bias, b1_bias, b2_bias, ln1_aff, ln2_aff = flags
    f32 = mybir.dt.float32
    i32 = mybir.dt.int32
    bf16 = mybir.dt.bfloat16
    AF = mybir.ActivationFunctionType
    OP = mybir.AluOpType
    AX = mybir.AxisListType

    nc = bacc.Bacc("TRN2", target_bir_lowering=False, debug=False)

    fp8 = mybir.dt.float8e4
    DR = mybir.MatmulPerfMode.DoubleRow
    x_d = nc.dram_tensor("x", [T, C], bf16, kind="ExternalInput")
    qkvp_d = {}
    for wn in ("wq", "wk", "wv", "wp"):
        for hl in ("h", "l"):
            qkvp_d[wn + hl] = nc.dram_tensor(
                wn + hl, [C, C], fp8, kind="ExternalInput"
            )
    w1h_d = nc.dram_tensor("w1h", [C, FF], fp8, kind="ExternalInput")
    w1l_d = nc.dram_tensor("w1l", [C, FF], fp8, kind="ExternalInput")
    w2h_d = nc.dram_tensor("w2h", [FF, C], fp8, kind="ExternalInput")
    w2l_d = nc.dram_tensor("w2l", [FF, C], fp8, kind="ExternalInput")
    cb_d = nc.dram_tensor("cbias", [P, 3], f32, kind="ExternalInput")
    tri_d = nc.dram_tensor("tri", [P, P], fp8, kind="ExternalInput")
    id_d = nc.dram_tensor("ident", [P, P], bf16, kind="ExternalInput")
    out_d = nc.dram_tensor("out", [T, C], bf16, kind="ExternalOutput")

    opt_d = {}
    if qk_bias:
        opt_d["bq"] = nc.dram_tensor("bq", [P, NKC], f32, kind="ExternalInput")
        opt_d["bk"] = nc.dram_tensor("bk", [P, NKC], f32, kind="ExternalInput")
    if v_bias:
        opt_d["bv"] = nc.dram_tensor("bv", [1, C], bf16, kind="ExternalInput")
    if p_bias:
        opt_d["bp"] = nc.dram_tensor("bp", [1, C], bf16, kind="ExternalInput")
    if b1_bias:
        opt_d["b1"] = nc.dram_tensor("b1", [P, NM], f32, kind="ExternalInput")
    if b2_bias:
        opt_d["b2"] = nc.dram_tensor("b2", [1, C], bf16, kind="ExternalInput")
    if ln1_aff:
        opt_d["g1"] = nc.dram_tensor("g1", [P, C], f32, kind="ExternalInput")
        opt_d["o1"] = nc.dram_tensor("o1", [P, C], f32, kind="ExternalInput")
    if ln2_aff:
        opt_d["g2"] = nc.dram_tensor("g2", [P, C], f32, kind="ExternalInput")
        opt_d["o2"] = nc.dram_tensor("o2", [P, C], f32, kind="ExternalInput")

    x_re = x_d.ap().rearrange("(t p) c -> p t c", p=P)
    out_re = out_d.ap().rearrange("(t p) c -> p t c", p=P)
    qkvp_re = {
        nm: d.ap().rearrange("(k p) m -> p k m", p=P)
        for nm, d in qkvp_d.items()
    }
    w1h_re = w1h_d.ap().rearrange("(k p) m -> p k m", p=P)
    w1l_re = w1l_d.ap().rearrange("(k p) m -> p k m", p=P)
    w2h_re = w2h_d.ap().rearrange("(k p) m -> p k m", p=P)
    w2l_re = w2l_d.ap().rearrange("(k p) m -> p k m", p=P)

    def kts_for(qc):
        # visible k-tiles for q-chunk qc (512-wide chunks)
        return range(4) if qc == 0 else range(8)

    with tile.TileContext(nc) as tc:
        import contextlib

        with contextlib.ExitStack() as ctx:
            cpool = ctx.enter_context(tc.tile_pool(name="const", bufs=1))
            xpool = ctx.enter_context(tc.tile_pool(name="xres", bufs=1))
            apool = ctx.enter_context(tc.tile_pool(name="act", bufs=1))
            spool = ctx.enter_context(tc.tile_pool(name="small", bufs=8))
            sqpool = ctx.enter_context(tc.tile_pool(name="sqscr", bufs=1))
            # one shared [128,512] fp32 psum tag for QKV / S^T / proj / MLP2 —
            # avoids pool-boundary serialization between phases
            mmps = ctx.enter_context(
                tc.tile_pool(name="mm512", bufs=5, space="PSUM")
            )
            # w1 stream pool lives at top level so its first DMAs aren't
            # gated on the attention-phase pools releasing SBUF
            w1p = ctx.enter_context(tc.tile_pool(name="w1p", bufs=9))
            w1pre = ctx.enter_context(tc.tile_pool(name="w1pre", bufs=1))
            u0p = ctx.enter_context(tc.tile_pool(name="u0", bufs=1))

            tri_sb = cpool.tile([P, P], fp8, tag="tri")
            nc.sync.dma_start(tri_sb[:], tri_d[:])
            id_sb = cpool.tile([P, P], bf16, tag="ident")
            nc.sync.dma_start(id_sb[:], id_d[:])
            cb_sb = cpool.tile([P, 3], f32, tag="cbias")
            nc.sync.dma_start(cb_sb[:], cb_d[:])
            magic_sb = cpool.tile([P, 1], i32, tag="magic")
            nc.vector.memset(magic_sb[:], 0x5F3759DF)
            need_ones_b = v_bias or p_bias or b2_bias
            if need_ones_b:
                ones_b = cpool.tile([1, P], bf16, tag="onesb")
                nc.gpsimd.memset(ones_b[:], 1.0)
            opt_sb = {}
            for nm, dd in opt_d.items():
                shp = list(dd.shape)
                dt_ = dd.dtype
                opt_sb[nm] = cpool.tile(shp, dt_, tag=nm)
                nc.sync.dma_start(opt_sb[nm][:], dd[:])

            x_sb = xpool.tile([P, NT, C], bf16, tag="x")
            for t in range(4):
                nc.sync.dma_start(x_sb[:, t, :], x_re[:, t, :])

            # ---------------- LayerNorm (token-major) + transpose ----------
            def ln_tile(dst_tok, t, affine, act_mean=False, mean_acc=None,
                        norm_act=False, act_sq=False):
                    xr = x_sb[:, t, :]
                    mu = spool.tile([P, 1], f32, tag="mu")
                    if mean_acc is not None:
                        # row-sums already accumulated by the residual-evict
                        nc.vector.tensor_add(
                            mu, mean_acc[:, 0:1], mean_acc[:, 1:2]
                        )
                        nc.vector.tensor_scalar_mul(mu, mu, 1.0 / C)
                    elif act_mean:
                        # mean via ACT Copy+accum (frees DVE on the startup
                        # critical path; Copy shares exp's LUT set)
                        cs = sqpool.tile([P, C], bf16, tag="sq")
                        nc.scalar.activation(cs, xr, AF.Copy, accum_out=mu)
                        nc.vector.tensor_scalar_mul(mu, mu, 1.0 / C)
                    else:
                        nc.vector.tensor_reduce(mu, xr, axis=AX.X, op=OP.add)
                        nc.vector.tensor_scalar_mul(mu, mu, 1.0 / C)
                    sq = sqpool.tile([P, C], bf16, tag="sq")
                    ss = spool.tile([P, 1], f32, tag="ss")
                    if mean_acc is not None and not act_sq:
                        # qc1-window LN: keep ACT free for exp — square on DVE
                        nc.vector.scalar_tensor_tensor(
                            sq, xr, 1.0, xr, op0=OP.mult, op1=OP.mult,
                            accum_out=ss,
                        )
                    else:
                        nc.scalar.activation(sq, xr, AF.Square, accum_out=ss)
                    var = spool.tile([P, 1], f32, tag="var")
                    musq = spool.tile([P, 1], f32, tag="musq")
                    nc.vector.tensor_mul(musq, mu, mu)
                    nc.vector.tensor_scalar_mul(var, ss, 1.0 / C)
                    nc.vector.tensor_sub(var, var, musq)
                    nc.vector.tensor_scalar_add(var, var, EPS)
                    # rstd = rsqrt(var) on DVE (bit-trick + 3 Newton steps):
                    # ACT Sqrt/Ln would thrash LUT-table loads against the
                    # attention exp stream (different act_func_sets)
                    rstd = spool.tile([P, 1], f32, tag="rstd")
                    ri = rstd[:].bitcast(i32)
                    nc.vector.tensor_single_scalar(
                        ri, var[:].bitcast(i32), 1, op=OP.arith_shift_right
                    )
                    nc.vector.tensor_sub(ri, magic_sb[:], ri)
                    nsq = spool.tile([P, 1], f32, tag="nsq")
                    for _ in range(2):
                        nc.vector.tensor_mul(nsq, rstd, rstd)
                        nc.vector.tensor_mul(nsq, nsq, var)
                        nc.vector.tensor_scalar(
                            nsq, nsq, -0.5, 1.5, op0=OP.mult, op1=OP.add
                        )
                        nc.vector.tensor_mul(rstd, rstd, nsq)
                    if affine is None and norm_act:
                        # xn = Identity(x*rstd + (-mu*rstd)) on ACT — used on
                        # alternate LN1 tiles to split the normalize pass
                        # across both engines (Identity shares exp's LUT set)
                        nmr = spool.tile([P, 1], f32, tag="nmr")
                        nc.vector.tensor_mul(nmr, mu, rstd)
                        nc.vector.tensor_scalar_mul(nmr, nmr, -1.0)
                        nc.scalar.activation(
                            dst_tok[:, t, :], xr, AF.Identity,
                            bias=nmr, scale=rstd,
                        )
                    elif affine is None:
                        nc.vector.tensor_scalar(
                            dst_tok[:, t, :], xr, mu, rstd,
                            op0=OP.subtract, op1=OP.mult,
                        )
                    else:
                        g_sb_, o_sb_ = affine
                        tmp = spool.tile([P, C], f32, tag="lntmp")
                        nc.vector.tensor_scalar(
                            tmp, xr, mu, rstd, op0=OP.subtract, op1=OP.mult
                        )
                        nc.vector.tensor_mul(tmp, tmp, g_sb_[:])
                        nc.vector.tensor_add(dst_tok[:, t, :], tmp, o_sb_[:])

            def transp_tile(dst_fT, src_tok, t, psum_pool, tag="tp"):
                for mc in range(NKC):
                    tp = psum_pool.tile([P, P], bf16, tag=tag,
                                        name=f"tp{t}_{mc}")
                    nc.tensor.transpose(
                        tp, src_tok[:, t, mc * P:(mc + 1) * P], id_sb[:]
                    )
                    nc.vector.tensor_copy(
                        dst_fT[:, mc, t * P:(t + 1) * P], tp
                    )

            def transp_one(dst_h, src_tok, t, psum_pool, tag="tp"):
                # batched transpose, single e4m3 evict (no lo residual)
                for half in range(2):
                    mc0 = half * 4
                    tp = psum_pool.tile([P, 512], bf16, tag=tag,
                                        name=f"t1_{t}_{half}")
                    for i in range(4):
                        nc.tensor.transpose(
                            tp[:, i * P:(i + 1) * P],
                            src_tok[:, t, (mc0 + i) * P:(mc0 + i + 1) * P],
                            id_sb[:],
                        )
                    nc.vector.tensor_copy(
                        dst_h[:, mc0:mc0 + 4, t * P:(t + 1) * P],
                        tp.rearrange("p (k q) -> p k q", q=P),
                    )

            def transp_split(dst_h, dst_l, src_tok, t, psum_pool, tag="tp"):
                # transpose 4 feature blocks into one [P,512] psum, then
                # evict as e4m3 hi + lo (lo = exact - hi)
                for half in range(2):
                    mc0 = half * 4
                    tp = psum_pool.tile([P, 512], bf16, tag=tag,
                                        name=f"tsp{t}_{half}")
                    for i in range(4):
                        nc.tensor.transpose(
                            tp[:, i * P:(i + 1) * P],
                            src_tok[:, t, (mc0 + i) * P:(mc0 + i + 1) * P],
                            id_sb[:],
                        )
                    tp3 = tp.rearrange("p (k q) -> p k q", q=P)
                    hsl = dst_h[:, mc0:mc0 + 4, t * P:(t + 1) * P]
                    nc.vector.tensor_copy(hsl, tp3)
                    nc.vector.tensor_sub(
                        dst_l[:, mc0:mc0 + 4, t * P:(t + 1) * P], tp3, hsl
                    )

            # ---------------- QKV + attention + proj -----------------------
            # q8: [*, mc, 2(dup), T]; k8: [*, mc, 2(hi|lo), T] so the S^T
            # DoubleRow pairs (k_hi,q)+(k_lo,q). v8/y8 single e4m3.
            with contextlib.ExitStack() as actx:
                qkvy = actx.enter_context(tc.tile_pool(name="qkvy", bufs=1))
                wpool = actx.enter_context(tc.tile_pool(name="wstream", bufs=2))
                # wq streams ahead of the second x half so the first QK
                # matmuls aren't DMA-gated
                wq_th = wpool.tile([P, NKC, C], fp8, tag="wh")
                nc.sync.dma_start(wq_th[:], qkvp_re["wqh"])
                for t in range(4, NT):
                    nc.sync.dma_start(x_sb[:, t, :], x_re[:, t, :])

                ln1_args = (opt_sb["g1"][:], opt_sb["o1"][:]) if ln1_aff else None
                ln2_args = (opt_sb["g2"][:], opt_sb["o2"][:]) if ln2_aff else None
                xn_tok = apool.tile([P, NT, C], bf16, tag="tok")
                xnT_h = apool.tile([P, NKC, T], fp8, tag="xTh")
                with tc.tile_pool(name="tpsum", bufs=2, space="PSUM") as tpp:
                    for t in range(NT):
                        ln_tile(xn_tok, t, ln1_args)
                        transp_one(xnT_h, xn_tok, t, tpp)

                q8 = qkvy.tile([P, NKC, 2, T], fp8, tag="q")
                k8 = qkvy.tile([P, NKC, 2, T], fp8, tag="k")
                v8 = qkvy.tile([P, NT, H, D + 1], fp8, tag="v")
                y8 = qkvy.tile([P, NKC, T], fp8, tag="y")
                nc.vector.memset(v8[:, :, :, D:D + 1], 1.0)

                def mm_ss_dr(ps, terms, qsl_m, qsl_x, stop_ok=True):
                    """DoubleRow kc-paired products: terms = [(w, x), ...]"""
                    first = True
                    for ti, (wa, xa) in enumerate(terms):
                        for kp in range(NKC // 2):
                            nc.tensor.matmul(
                                ps,
                                wa[:, 2 * kp:2 * kp + 2, qsl_m],
                                xa[:, 2 * kp:2 * kp + 2, qsl_x],
                                start=first,
                                stop=stop_ok and (ti == len(terms) - 1)
                                and (kp == NKC // 2 - 1),
                                perf_mode=DR,
                            )
                            first = False

                # Q and K (feature-major); n2-outer so the first token half's
                # xnT transposes unblock matmuls early
                def qk_chunk(which, dst, wht, n2, m):
                    if True:
                        if True:
                            ps = mmps.tile([P, 512], f32, tag="S")
                            qsl = slice(n2 * 512, (n2 + 1) * 512)
                            mm_ss_dr(
                                ps,
                                ((wht, xnT_h),),
                                slice(m * P, (m + 1) * P), qsl,
                            )
                            d0 = dst[:, m, 0, qsl]
                            d1 = dst[:, m, 1, qsl]
                            if qk_bias:
                                bias_nm = "bq" if which == 0 else "bk"
                                sc = sqpool.tile([P, 512], bf16, tag="qksc")
                                nc.scalar.activation(
                                    sc, ps, AF.Identity,
                                    bias=opt_sb[bias_nm][:, m:m + 1],
                                    scale=1.0 / 32,
                                )
                                nc.vector.tensor_copy(d0, sc)
                                if which == 0:
                                    nc.gpsimd.tensor_copy(d1, d0)
                                else:
                                    nc.vector.tensor_sub(d1, sc, d0)
                            else:
                                nc.scalar.activation(d0, ps, AF.Identity,
                                                     scale=1.0 / 32)
                                if which == 0:
                                    # q duplicated across both DR slots
                                    nc.gpsimd.tensor_copy(d1, d0)
                                else:
                                    # k_lo = exact - k_hi
                                    nc.vector.scalar_tensor_tensor(
                                        d1, ps, 1.0 / 32, d0,
                                        op0=OP.mult, op1=OP.subtract,
                                    )

                wk_th = wpool.tile([P, NKC, C], fp8, tag="wh")
                nc.sync.dma_start(wk_th[:], qkvp_re["wkh"])
                for n2 in (0, 1):
                    for m in range(NKC):
                        qk_chunk(0, q8, wq_th, n2, m)
                # K n2=1 is first consumed by qc1 — deferred into qc0 fills
                for m in range(NKC):
                    qk_chunk(1, k8, wk_th, 0, m)

                # V (token-major, strided into per-head 65-wide slots).
                # n2=0 (heads 0..7) now; n2=1 groups are deferred into the
                # ACT-bound qc1 attention stream as PE filler (heads 8..15
                # aren't consumed until the 9th qc1 pair).
                wvh_sb = wpool.tile([P, NKC, C], fp8, tag="wh")
                nc.sync.dma_start(wvh_sb[:], qkvp_re["wvh"])

                def emit_v(t, n2):
                    ps = mmps.tile([P, 512], f32, tag="S")
                    nsl = slice(n2 * 512, (n2 + 1) * 512)
                    tsl = slice(t * P, (t + 1) * P)
                    first = True
                    for kp in range(NKC // 2):
                        nc.tensor.matmul(
                            ps,
                            xnT_h[:, 2 * kp:2 * kp + 2, tsl],
                            wvh_sb[:, 2 * kp:2 * kp + 2, nsl],
                            start=first,
                            stop=(kp == NKC // 2 - 1) and not v_bias,
                            perf_mode=DR,
                        )
                        first = False
                    if v_bias:
                        nc.tensor.matmul(
                            ps, ones_b[:],
                            opt_sb["bv"][:, n2 * 512:(n2 + 1) * 512],
                            start=False, stop=True,
                        )
                    nc.scalar.activation(
                        v8[:, t, n2 * 8:(n2 + 1) * 8, 0:D],
                        ps.rearrange("p (h d) -> p h d", d=D),
                        AF.Identity, scale=1.0 / 32,
                    )

                for t in range(4):
                    emit_v(t, 0)

                # ---- attention (qc0 first) with V t4..7 filling the qc0
                # stream and proj/LN2 for t0..3 filling the qc1 stream ----
                h_tok = apool.tile([P, NT, C], bf16, tag="tok")
                hT_h = apool.tile([P, NKC, T], fp8, tag="fTh")
                hT_l = apool.tile([P, NKC, T], fp8, tag="fTl")

                def emit_proj(t, n2):
                    # proj is y8(native) @ Wp(hi+lo), mc-paired DoubleRow
                    ps = mmps.tile([P, 512], f32, tag="S")
                    nsl = slice(n2 * 512, (n2 + 1) * 512)
                    tsl = slice(t * P, (t + 1) * P)
                    first = True
                    for kp in range(NKC // 2):
                        nc.tensor.matmul(
                            ps,
                            y8[:, 2 * kp:2 * kp + 2, tsl],
                            wph_sb[:, 2 * kp:2 * kp + 2, nsl],
                            start=first,
                            stop=(kp == NKC // 2 - 1) and not p_bias,
                            perf_mode=DR,
                        )
                        first = False
                    if p_bias:
                        nc.tensor.matmul(
                            ps, ones_b[:],
                            opt_sb["bp"][:, n2 * 512:(n2 + 1) * 512],
                            start=False, stop=True,
                        )
                    xsl = x_sb[:, t, n2 * 512:(n2 + 1) * 512]
                    if t not in proj_acc:
                        proj_acc[t] = spool.tile([P, 2], f32, tag="pacc", name=f"pacc{t}")
                    # x1 = ps/32 + x, with the row-sum accumulated on the side
                    # so LN2 doesn't need its own mean-reduction pass
                    nc.vector.scalar_tensor_tensor(
                        xsl, ps, 1.0 / 32, xsl, op0=OP.mult, op1=OP.add,
                        accum_out=proj_acc[t][:, n2:n2 + 1],
                    )

                proj_acc = {}
                with (
                    tc.tile_pool(name="epool", bufs=2) as epool,
                    tc.tile_pool(name="attpy", bufs=2, space="PSUM") as yps,
                    tc.tile_pool(name="tpsum2", bufs=1, space="PSUM") as tp2,
                    tc.tile_pool(name="attsb", bufs=2) as asb,
                ):
                    e_tiles = {}

                    def emit_s_kt(h, qc, e_t, kt):
                        po = (h % 2) * 64
                        mc = h // 2
                        qsl = slice(qc * 512, (qc + 1) * 512)
                        s_ps = mmps.tile([P, 512], f32, tag="S")
                        # DoubleRow slots: (k_hi, q) + (k_lo, q-dup)
                        nc.tensor.matmul(
                            s_ps,
                            k8[po:po + 64, mc, :, kt * P:(kt + 1) * P],
                            q8[po:po + 64, mc, :, qsl],
                            start=True, stop=True,
                            perf_mode=DR,
                        )
                        w = 0
                        diag = kt >= 2 and kt // 4 == qc
                        if diag:
                            w = kt * P - qc * 512
                            if w > 0:
                                nc.gpsimd.memset(e_t[:, kt, 0:w], 0.0)
                        bias = cb_sb[:, kt:kt + 1] if kt < 2 else cb_sb[:, 2:3]
                        nc.scalar.activation(
                            e_t[:, kt, w:512], s_ps[:, w:512], AF.Exp,
                            bias=bias, scale=0.125,
                        )
                        if diag:
                            # zero the upper-triangle of the diagonal block
                            # post-exp (0/1 mask multiply on gpsimd)
                            nc.gpsimd.tensor_mul(
                                e_t[:, kt, w:w + P],
                                e_t[:, kt, w:w + P],
                                tri_sb[:],
                            )

                    def emit_sav(cur, prev):
                        """S matmuls of pair `cur` interleaved with AV
                        DoubleRow kt-pair matmuls of pair `prev` — spreads
                        PSUM slot demand and keeps exp lead ahead of AV."""
                        if cur is not None:
                            e_cur = epool.tile([P, NKC, 512], fp8, tag="E")
                            e_tiles[cur] = e_cur
                            skts = list(kts_for(cur[1]))
                        else:
                            skts = []
                        akp = (len(kts_for(prev[1])) // 2) if prev else 0
                        y_ps = None
                        if prev:
                            h, qc = prev
                            e_prev = e_tiles.pop(prev)
                            y_ps = yps.tile([D + 1, 512], f32, tag="Y")
                        for idx in range(max(len(skts), 2 * akp)):
                            if idx < len(skts):
                                emit_s_kt(cur[0], cur[1], e_cur, skts[idx])
                            if idx % 2 == 1 and idx // 2 < akp:
                                kp = idx // 2
                                nc.tensor.matmul(
                                    y_ps,
                                    v8[:, 2 * kp:2 * kp + 2, prev[0], :],
                                    e_prev[:, 2 * kp:2 * kp + 2, :],
                                    start=(kp == 0),
                                    stop=(kp == akp - 1),
                                    perf_mode=DR,
                                )
                        if prev:
                            emit_norm(prev[0], prev[1], y_ps)

                    def emit_norm(h, qc, y_ps):
                        po = (h % 2) * 64
                        mc = h // 2
                        qsl = slice(qc * 512, (qc + 1) * 512)
                        r_sb = asb.tile([D + 1, 512], f32, tag="r")
                        nc.vector.reciprocal(
                            r_sb[D:D + 1, :], y_ps[D:D + 1, :]
                        )
                        # partition_broadcast HW reads the tile's partition 0
                        # (AP partition offset ignored) — bounce row 64 -> 0
                        r0_sb = asb.tile([1, 512], f32, tag="r0")
                        nc.sync.dma_start(r0_sb[:], r_sb[D:D + 1, :])
                        bcs = asb.tile([64, 512], f32, tag="bcs")
                        nc.gpsimd.partition_broadcast(bcs, r0_sb[:])
                        if po == 0:
                            # even heads are already lane-aligned with the
                            # y8 destination: write directly, no DMA shift
                            nc.vector.tensor_mul(
                                y8[0:64, mc, qsl], y_ps[0:D, :], bcs
                            )
                        else:
                            yt = asb.tile([64, 512], fp8, tag="yt")
                            nc.vector.tensor_mul(yt, y_ps[0:D, :], bcs)
                            nc.sync.dma_start(y8[po:po + 64, mc, qsl], yt)

                    # qc=0 first: its stream is filled with the deferred
                    # V t4..7 chunks; the ACT-bound qc=1 stream then takes
                    # proj/LN2 for t0..3 plus 16 MLP1 n2=0 chunks whose raw
                    # psums park in u0 (gelu deferred past attention).
                    u0 = u0p.tile([P, 24, 512], bf16, tag="u0")
                    w1pre_t = {}

                    def mlp1_u_chunk(m):
                        w1th = w1p.tile([P, NKC, P], fp8, tag="w1h")
                        w1tl = w1p.tile([P, NKC, P], fp8, tag="w1l")
                        nc.sync.dma_start(
                            w1th[:], w1h_re[:, :, m * P:(m + 1) * P])
                        nc.sync.dma_start(
                            w1tl[:], w1l_re[:, :, m * P:(m + 1) * P])
                        if m >= 15:
                            # ring depth 7: these tiles stay resident for
                            # the post-attention n2=1 pass (no re-DMA)
                            w1pre_t[m] = (w1th, w1tl)
                        ps = mmps.tile([P, 512], f32, tag="S")
                        mm_ss_dr(
                            ps,
                            ((w1th, hT_h), (w1th, hT_l), (w1tl, hT_h)),
                            slice(0, P), slice(0, 512),
                        )
                        nc.vector.tensor_copy(u0[:, m, :], ps)

                    pairs = [(h, 0) for h in range(H)] + \
                            [(h, 1) for h in range(H)]
                    for i in range(len(pairs) + 1):
                        cur = pairs[i] if i < len(pairs) else None
                        prev = pairs[i - 1] if i > 0 else None
                        emit_sav(cur, prev)
                        if i > 0:
                            j = i - 1
                            if i == 16:
                                wph_sb = wpool.tile([P, NKC, C], fp8,
                                                    tag="wh", name="wpht")
                                nc.sync.dma_start(wph_sb[:], qkvp_re["wph"])
                            if j < H:
                                # qc0 stream: V t0..3 n2=1 (even j, needed
                                # from pair 8), V t4..7 (odd j), K n2=1
                                # (even j >= 8, needed only by qc1)
                                if j % 2 == 0 and j < 8:
                                    emit_v(j // 2, 1)
                                elif j % 2 == 1:
                                    jj = j // 2
                                    emit_v(4 + jj // 2, jj % 2)
                                else:
                                    mk = j - 8
                                    qk_chunk(1, k8, wk_th, 1, mk)
                                    qk_chunk(1, k8, wk_th, 1, mk + 1)
                            else:
                                jj = j - H
                                if jj < NT:
                                    emit_proj(jj // 2, jj % 2)
                                    if jj % 2 == 1:
                                        t = jj // 2
                                        ln_tile(h_tok, t, ln2_args,
                                                mean_acc=proj_acc.pop(t))
                                        transp_split(hT_h, hT_l, h_tok, t,
                                                     tp2)
                                else:
                                    mlp1_u_chunk(3 * (jj - NT))
                                    mlp1_u_chunk(3 * (jj - NT) + 1)
                                    mlp1_u_chunk(3 * (jj - NT) + 2)
                                    if jj == 15:
                                        # w1 for m<8's n2=1 streams in now,
                                        # while the DMA engines are idle
                                        for mi in range(8):
                                            th = w1pre.tile(
                                                [P, NKC, P], fp8,
                                                tag=f"w1pa{mi}",
                                                name=f"w1pa{mi}")
                                            tl = w1pre.tile(
                                                [P, NKC, P], fp8,
                                                tag=f"w1pb{mi}",
                                                name=f"w1pb{mi}")
                                            nc.sync.dma_start(
                                                th[:],
                                                w1h_re[:, :, mi * P:(mi + 1) * P])
                                            nc.sync.dma_start(
                                                tl[:],
                                                w1l_re[:, :, mi * P:(mi + 1) * P])
                                            w1pre_t[mi] = (th, tl)

                    for t in range(4, NT):
                        emit_proj(t, 0)
                        emit_proj(t, 1)
                        ln_tile(h_tok, t, ln2_args,
                                mean_acc=proj_acc.pop(t), act_sq=True)
                        transp_split(hT_h, hT_l, h_tok, t, tp2)

            # ---------------- MLP (fp8 DoubleRow, both operands hi+lo) ------
            # W1 pre-scaled x32 (unit std), W2 x64; psum scales undone at
            # evict (gelu scale=1/32, final stt scale=1/64).
            with contextlib.ExitStack() as mctx:
                gpool = mctx.enter_context(tc.tile_pool(name="g", bufs=1))
                gscr = mctx.enter_context(tc.tile_pool(name="gscr", bufs=3))
                g_h = gpool.tile([P, NM, T], fp8, tag="gh")
                # only m>=16 keeps a lo residual (hi+lo split); m<16 is
                # native e4m3 straight from ACT gelu (no cast/sub/lo-term)
                g_l = gpool.tile([P, 16, T], fp8, tag="gl")

                def mm_ss(ps, wh, wl, xh, xl, npair=NKC // 2):
                    """12 DoubleRow matmuls: hh, lh, hl over 4 kc-pairs."""
                    first = True
                    for wa, xa in ((wh, xh), (wh, xl), (wl, xh)):
                        for kp in range(npair):
                            nc.tensor.matmul(
                                ps,
                                wa[:, 2 * kp:2 * kp + 2, :],
                                xa[:, 2 * kp:2 * kp + 2, :],
                                start=first,
                                stop=(wa is wl) and (kp == npair - 1),
                                perf_mode=DR,
                            )
                            first = False

                def g_evict(m, qsl, src, b1c):
                    gsl_h = g_h[:, m, qsl]
                    if m < 16:
                        # native e4m3: one direct ACT gelu, no residual
                        if b1c is not None:
                            nc.scalar.activation(gsl_h, src, AF.Gelu,
                                                 bias=b1c, scale=1.0 / 32)
                        else:
                            nc.scalar.activation(gsl_h, src, AF.Gelu,
                                                 scale=1.0 / 32)
                        return
                    # hi+lo: gelu to bf16 scratch; hi cast on Pool, lo on DVE
                    gs = gscr.tile([P, 512], bf16, tag="gs")
                    if b1c is not None:
                        nc.scalar.activation(gs, src, AF.Gelu,
                                             bias=b1c, scale=1.0 / 32)
                    else:
                        nc.scalar.activation(gs, src, AF.Gelu,
                                             scale=1.0 / 32)
                    nc.gpsimd.tensor_copy(gsl_h, gs)
                    nc.vector.tensor_sub(g_l[:, m - 16, qsl], gs, gsl_h)

                for m in list(range(15, 24)) + list(range(15)) + \
                        list(range(24, NM)):
                    if m in w1pre_t:
                        w1th, w1tl = w1pre_t[m]
                    else:
                        w1th = w1p.tile([P, NKC, P], fp8, tag="w1h")
                        w1tl = w1p.tile([P, NKC, P], fp8, tag="w1l")
                        nc.sync.dma_start(
                            w1th[:], w1h_re[:, :, m * P:(m + 1) * P])
                        nc.sync.dma_start(
                            w1tl[:], w1l_re[:, :, m * P:(m + 1) * P])
                    b1c = opt_sb["b1"][:, m:m + 1] if b1_bias else None
                    for n2 in (1,) if m < 24 else (1, 0):
                        ps = mmps.tile([P, 512], f32, tag="S")
                        qsl = slice(n2 * 512, (n2 + 1) * 512)
                        mm_ss(ps, w1th, w1tl,
                              hT_h[:, :, qsl], hT_l[:, :, qsl])
                        g_evict(m, qsl, ps, b1c)
                    if m < 24:
                        # n2=0 raw psum was parked in u0 during attention
                        g_evict(m, slice(0, 512), u0[:, m, :], b1c)

                with (
                    tc.tile_pool(name="w2p", bufs=2) as w2p,
                    tc.tile_pool(name="outp", bufs=4) as outp,
                ):
                    for n4 in range(4):
                        nsl = slice(n4 * 256, (n4 + 1) * 256)
                        w2th = w2p.tile([P, NM, 256], fp8, tag="w2h")
                        w2tl = w2p.tile([P, NM, 256], fp8, tag="w2l")
                        nc.sync.dma_start(w2th[:], w2h_re[:, :, nsl])
                        nc.sync.dma_start(w2tl[:], w2l_re[:, :, nsl])
                        for t in range(NT):
                            ps = mmps.tile([P, 256], f32, tag="S")
                            tsl = slice(t * P, (t + 1) * P)
                            first = True
                            for ga, wa in ((g_h, w2th), (g_h, w2tl)):
                                for kp in range(NM // 2):
                                    nc.tensor.matmul(
                                        ps,
                                        ga[:, 2 * kp:2 * kp + 2, tsl],
                                        wa[:, 2 * kp:2 * kp + 2, :],
                                        start=first, stop=False,
                                        perf_mode=DR,
                                    )
                                    first = False
                            for kp in range(8, NM // 2):
                                nc.tensor.matmul(
                                    ps,
                                    g_l[:, 2 * (kp - 8):2 * (kp - 8) + 2,
                                        tsl],
                                    w2th[:, 2 * kp:2 * kp + 2, :],
                                    start=False,
                                    stop=(kp == NM // 2 - 1)
                                    and not b2_bias,
                                    perf_mode=DR,
                                )
                            if b2_bias:
                                nc.tensor.matmul(
                                    ps, ones_b[:], opt_sb["b2"][:, nsl],
                                    start=False, stop=True,
                                )
                            oc = outp.tile([P, 256], bf16, tag="oc")
                            nc.vector.scalar_tensor_tensor(
                                oc, ps, 1.0 / 64, x_sb[:, t, nsl],
                                op0=OP.mult, op1=OP.add,
                            )
                            nc.sync.dma_start(out_re[:, t, nsl], oc)

    nc.compile()
    return nc


def _host_aux(cond_mask):
    """Build per-batch cond bias [P, 2] and shared tri [P, 640] / identity."""
    counts = np.asarray(cond_mask).sum(axis=-1).astype(np.int64)  # [B]
    cbias = []
    for b in range(B):
        vec = np.full(COND_LEN, -ESHIFT, np.float32)
        vec[counts[b]:] = NEG
        cb = np.empty((P, 3), np.float32)
        cb[:, 0:2] = vec.reshape(2, P).T
        cb[:, 2] = -ESHIFT
        cbias.append(cb)
    kk = np.arange(P)[:, None]
    qq = np.arange(P)[None, :]
    tri = (qq >= kk).astype(E4M3)
    ident = np.eye(P, dtype=BF16)
    return cbias, tri, ident


def kernel(**inputs):
    from concourse.bass_utils import run_bass_kernel_spmd

    x = np.asarray(inputs["x"], np.float32)
    assert x.shape == (B, T, C)
    assert int(inputs["cond_len"]) == COND_LEN
    assert int(inputs["token_len"]) == TOKEN_LEN

    f32 = np.float32
    Wq, Wk, Wv, Wp = (np.asarray(inputs[k], f32) for k in ("Wq", "Wk", "Wv", "Wp"))
    W1, W2 = np.asarray(inputs["W1"], f32), np.asarray(inputs["W2"], f32)
    bq, bk, bv, bp = (np.asarray(inputs[k], f32) for k in ("bq", "bk", "bv", "bp"))
    b1, b2 = np.asarray(inputs["b1"], f32), np.asarray(inputs["b2"], f32)
    g1, o1 = np.asarray(inputs["ln1_g"], f32), np.asarray(inputs["ln1_b"], f32)
    g2, o2 = np.asarray(inputs["ln2_g"], f32), np.asarray(inputs["ln2_b"], f32)

    flags = (
        bool(bq.any() or bk.any()),
        bool(bv.any()),
        bool(bp.any()),
        bool(b1.any()),
        bool(b2.any()),
        bool((g1 != 1).any() or o1.any()),
        bool((g2 != 1).any() or o2.any()),
    )
    if flags not in _BUILD_CACHE:
        _BUILD_CACHE[flags] = _build(flags)
    nc = _BUILD_CACHE[flags]
    qk_bias, v_bias, p_bias, b1_bias, b2_bias, ln1_aff, ln2_aff = flags

    cbias, tri, ident = _host_aux(inputs["cond_mask"])
    w1h, w1l = _split8(W1, 32.0)
    w2h, w2l = _split8(W2, 64.0)
    shared = {
        "w1h": w1h, "w1l": w1l, "w2h": w2h, "w2l": w2l,
        "tri": tri, "ident": ident,
    }
    for wn, W in (("wq", Wq), ("wk", Wk), ("wv", Wv), ("wp", Wp)):
        hi, lo = _split8(W, 32.0)
        shared[wn + "h"], shared[wn + "l"] = hi, lo
    if qk_bias:
        shared["bq"] = np.ascontiguousarray(bq.reshape(NKC, P).T)
        shared["bk"] = np.ascontiguousarray(bk.reshape(NKC, P).T)
    if v_bias:
        shared["bv"] = (bv * 32.0).reshape(1, C).astype(BF16)
    if p_bias:
        shared["bp"] = (bp * 32.0).reshape(1, C).astype(BF16)
    if b1_bias:
        shared["b1"] = np.ascontiguousarray(b1.reshape(NM, P).T)
    if b2_bias:
        shared["b2"] = (b2 * 64.0).reshape(1, C).astype(BF16)
    if ln1_aff:
        shared["g1"] = np.broadcast_to(g1, (P, C)).copy()
        shared["o1"] = np.broadcast_to(o1, (P, C)).copy()
    if ln2_aff:
        shared["g2"] = np.broadcast_to(g2, (P, C)).copy()
        shared["o2"] = np.broadcast_to(o2, (P, C)).copy()

    in_maps = [dict(shared, x=x[b].astype(BF16), cbias=cbias[b]) for b in range(B)]
    try:
        res = run_bass_kernel_spmd(nc, in_maps, list(range(B)),
                                   trace=kernel._trace)
    except ModuleNotFoundError:
        # ntff profiling hook unavailable in this container; run untraced
        res = run_bass_kernel_spmd(nc, in_maps, list(range(B)), trace=False)
    kernel._last_results = res
    out = np.stack([res.results[b]["out"] for b in range(B)], axis=0)
    return out.astype(np.float32)


kernel._trace = False
kernel._last_results = None

